# revision 27
# baseline (speedup 1.0000x reference)
"""Trainium2 Bass kernel for nn_AutoregressiveFormulaDecoder.

2-layer GRU decoder with teacher forcing, fused MLP head.
Data-parallel over 8 NeuronCores (1024 batch rows per core).

Device layout: "transposed" — features on SBUF partitions, batch on the
free dimension — so weights are the PE-stationary operand and per-feature
biases are per-partition ACT biases.

Per step t (49 steps), per batch chunk of 512:
  - gi0 comes from a one-hot matmul against emb2 = emb @ W_ih0.T
    (one-hot planes are built on host from the integer tokens).
  - r/z gates: gi and gh matmuls ACCUMULATE in the same PSUM bank, then
    one Sigmoid activation with fused per-partition bias reads PSUM.
  - n gate: i_n and h_n kept in separate PSUM banks; fused DVE
    (h_n + b_hn) * r, + i_n, then Tanh with fused bias.
  - h' = n + z*(h - n) on DVE.
  - Head fused per step: relu(W1 @ h1') then W2 @ ... -> logits tile,
    DMA'd straight to DRAM.

All matmuls run in float32r (full f32 storage, 1 cycle/row on PE for
moving dim >= 256) via AP bitcast — no precision-losing casts.
"""

import numpy as np

VOCAB = 148
START_IDX = 1
LATENT = 32
HID = 256
G3 = 3 * HID  # 768
B = 8192
T = 50
NSTEPS = T - 1  # 49
NCORES = 8
BL = B // NCORES  # 1024 batch rows per core
CH = 512          # batch chunk (one PSUM bank of f32)
NCH = BL // CH    # 2


# packed constant layout: name -> (col offset, col width); all float32 columns
_PACK_SPEC = [
    ("emb2a", G3), ("emb2b", G3),
    ("whh0k0", G3), ("whh0k1", G3),
    ("wih1k0", G3), ("wih1k1", G3),
    ("whh1k0", G3), ("whh1k1", G3),
    ("w1k0", HID), ("w1k1", HID),
    ("w2k0", VOCAB), ("w2k1", VOCAB),
    ("wlat", 2 * HID), ("zT", BL), ("biases", 24),
]
PACK_OFF = {}
_o = 0
for _n, _w in _PACK_SPEC:
    PACK_OFF[_n] = (_o, _w)
    _o += _w
PACK_COLS = _o


def _build_graph(n_steps=NSTEPS, zero_bias=True, fp8_embed=True):
    import concourse.bass as bass
    import concourse.bacc as bacc
    import concourse.mybir as mybir
    import concourse.tile as tile

    F32 = mybir.dt.float32
    BF16 = mybir.dt.bfloat16
    FP8 = mybir.dt.float8e4
    DR = mybir.MatmulPerfMode.DoubleRow
    AF = mybir.ActivationFunctionType
    OP = mybir.AluOpType

    nc = bacc.Bacc()

    if fp8_embed:
        oh_d = nc.declare_dram_parameter("oh", [n_steps, 74, 2 * BL], FP8,
                                         isOutput=False)
        emb2dr_d = nc.declare_dram_parameter("emb2dr", [74, 2 * G3], FP8,
                                             isOutput=False)
    else:
        oh_d = nc.declare_dram_parameter("oh", [n_steps, VOCAB, BL], BF16,
                                         isOutput=False)
    wpack_d = nc.declare_dram_parameter("wpack", [128, PACK_COLS], BF16, isOutput=False)
    out_d = nc.declare_dram_parameter("out", [n_steps, VOCAB, BL], F32, isOutput=True)

    with tile.TileContext(nc) as tc:
        with (
            tc.tile_pool(name="const", bufs=1) as cpool,
            tc.tile_pool(name="io", bufs=4) as iopool,
            tc.tile_pool(name="work", bufs=2) as wpool,
            tc.tile_pool(name="psum", bufs=1, space="PSUM") as ppool,
        ):
            # ---- one DMA for every constant ----
            wpk = cpool.tile([128, PACK_COLS], BF16)
            nc.sync.dma_start(wpk[:], wpack_d[:, :])
            if fp8_embed:
                emb2dr = cpool.tile([74, 2 * G3], FP8)
                nc.sync.dma_start(emb2dr[:], emb2dr_d[:, :])

            def P(name, rows=128):
                o, w = PACK_OFF[name]
                return wpk[0:rows, o:o + w]

            emb2a = P("emb2a")
            emb2b = P("emb2b", rows=VOCAB - 128)
            whh0 = [P("whh0k0"), P("whh0k1")]
            wih1 = [P("wih1k0"), P("wih1k1")]
            whh1 = [P("whh1k0"), P("whh1k1")]
            w1 = [P("w1k0"), P("w1k1")]
            w2 = [P("w2k0"), P("w2k1")]
            wlat = P("wlat", rows=LATENT)
            zT = P("zT", rows=LATENT)

            def bias_ap(col, rows=128):
                o, _ = PACK_OFF["biases"]
                return wpk[0:rows, o + col:o + col + 1]

            def mm(pt, lhsT, rhs, start, stop):
                nc.tensor.matmul(pt, lhsT, rhs, start=start, stop=stop)

            # ---- init hidden state: hT = W_lat @ zT + b_lat ----
            h0 = [None] * NCH   # wide [128, (k,512)] bf16 per chunk
            h1 = [None] * NCH
            for c in range(NCH):
                cs = slice(c * CH, (c + 1) * CH)
                h0[c] = wpool.tile([128, 2 * CH], BF16, tag="h0", bufs=4,
                                   name=f"h0i{c}")
                h1[c] = wpool.tile([128, 2 * CH], BF16, tag="h1", bufs=4,
                                   name=f"h1i{c}")
                for m in range(4):
                    ph = ppool.tile([128, CH], F32, tag="pn", bufs=4,
                                    name=f"pinit{c}_{m}")
                    mm(ph[:], wlat[:, m * 128:(m + 1) * 128], zT[:, cs],
                       True, True)
                    dst = (h0[c] if m < 2 else h1[c])
                    nc.scalar.activation(dst[:, (m % 2) * CH:(m % 2 + 1) * CH],
                                         ph[:], AF.Identity,
                                         bias=bias_ap(16 + m))

            def emit_head(t, h1s):
                for c in range(NCH):
                    cs = slice(c * CH, (c + 1) * CH)
                    hdd = wpool.tile([128, 2 * CH], BF16, tag="hdd", bufs=4,
                                     name=f"hdd{t}_{c}")
                    phds = []
                    for m in range(2):
                        ms = slice(m * 128, (m + 1) * 128)
                        phd = ppool.tile([128, CH], F32, tag="pr", bufs=2,
                                         name=f"phd{t}{c}{m}")
                        mm(phd[:], w1[0][:, ms], h1s[c][:, 0:CH], True, False)
                        mm(phd[:], w1[1][:, ms], h1s[c][:, CH:2 * CH], False, True)
                        phds.append(phd)
                    for m in range(2):
                        nc.scalar.activation(hdd[:, m * CH:(m + 1) * CH],
                                             phds[m][:], AF.Relu,
                                             bias=bias_ap(12 + m))
                    pl0 = ppool.tile([128, CH], F32, tag="pn", bufs=4,
                                     name=f"pl0{t}{c}")
                    mm(pl0[:], w2[0][:, 0:128], hdd[:, 0:CH], True, False)
                    mm(pl0[:], w2[1][:, 0:128], hdd[:, CH:2 * CH], False, True)
                    pl1 = ppool.tile([VOCAB - 128, CH], F32, tag="pn", bufs=4,
                                     name=f"pl1{t}{c}")
                    mm(pl1[:], w2[0][:, 128:VOCAB], hdd[:, 0:CH], True, False)
                    mm(pl1[:], w2[1][:, 128:VOCAB], hdd[:, CH:2 * CH], False, True)
                    lg0 = iopool.tile([128, CH], F32, tag="lg0",
                                      name=f"lg0{t}{c}")
                    lg1 = iopool.tile([VOCAB - 128, CH], F32, tag="lg1",
                                      name=f"lg1{t}{c}")
                    if zero_bias:
                        nc.vector.tensor_scalar_add(lg0[:], pl0[:], 0.0)
                        nc.vector.tensor_scalar_add(lg1[:], pl1[:], 0.0)
                    else:
                        nc.scalar.activation(lg0[:], pl0[:], AF.Identity,
                                             bias=bias_ap(14))
                        nc.scalar.activation(lg1[:], pl1[:], AF.Identity,
                                             bias=bias_ap(15, rows=VOCAB - 128))
                    nc.sync.dma_start(out_d[t, 0:128, cs], lg0[:])
                    nc.sync.dma_start(out_d[t, 128:VOCAB, cs], lg1[:])

            pending_head = None

            # ---- time loop ----
            # Emission order = per-engine execution order. Emit chunk c's
            # matmuls, then its gate chain; chunk c+1's matmuls fill the PE
            # while chunk c's ACT/DVE chain runs. z-gate PSUM groups are
            # emitted last within a chunk (z is needed late) to cut peak
            # PSUM pressure.
            for t in range(n_steps):
                ohs = []
                for c in range(NCH):
                    cs = slice(c * CH, (c + 1) * CH)
                    if fp8_embed:
                        ohc = iopool.tile([74, 2 * CH], FP8, tag="oha",
                                          name=f"oh{t}_{c}")
                        nc.sync.dma_start(
                            ohc[:], oh_d[t].rearrange(
                                "k (j b) -> k j b", j=2)[:, :, cs])
                        ohs.append(ohc)
                    else:
                        oha = iopool.tile([128, CH], BF16, tag="oha",
                                          name=f"oha{t}_{c}")
                        nc.sync.dma_start(oha[:], oh_d[t, 0:128, cs])
                        ohb = iopool.tile([VOCAB - 128, CH], BF16, tag="ohb",
                                          name=f"ohb{t}_{c}")
                        nc.sync.dma_start(ohb[:], oh_d[t, 128:VOCAB, cs])
                        ohs.append((oha, ohb))

                h0new = [None] * NCH
                for layer in range(2):
                    if layer == 1 and pending_head is not None:
                        emit_head(*pending_head)
                        pending_head = None
                    if layer == 0:
                        wh = whh0
                        sigc, tanc, bhnc = 0, (4, 5), (20, 21)
                    else:
                        wh = whh1
                        sigc, tanc, bhnc = 6, (10, 11), (22, 23)

                    for c in range(NCH):
                        hprev = h0[c] if layer == 0 else h1[c]
                        use_dr = fp8_embed and layer == 0
                        if layer == 0:
                            if not fp8_embed:
                                ia, ib = emb2a, emb2b
                                ra, rb = ohs[c]
                            else:
                                oh_rhs = ohs[c].rearrange("k (j b) -> k j b", j=2)
                        else:
                            ia, ib = wih1[0], wih1[1]
                            ra = h0new[c][:, 0:CH]
                            rb = h0new[c][:, CH:2 * CH]

                        def mm_gi(pg, gs, start, stop):
                            # gi contribution for gate rows gs
                            if use_dr:
                                lhs = emb2dr.rearrange(
                                    "k (j m) -> k j m", j=2)[:, :, gs]
                                nc.tensor.matmul(pg, lhs, oh_rhs,
                                                 start=start, stop=stop,
                                                 perf_mode=DR)
                            else:
                                mm(pg, ia[:, gs], ra, start, False)
                                mm(pg, ib[:, gs], rb, False, stop)

                        def grp4(pg, gs):
                            mm(pg[:], wh[0][:, gs], hprev[:, 0:CH], True, False)
                            mm(pg[:], wh[1][:, gs], hprev[:, CH:2 * CH], False, False)
                            mm_gi(pg[:], gs, False, True)

                        # r first, then n-gate psum, z last
                        pr, pin, phn, pz = [], [], [], []
                        for g in range(2):
                            pg = ppool.tile([128, CH], F32, tag="pr", bufs=2,
                                            name=f"pr{t}{c}{layer}{g}")
                            grp4(pg, slice(g * 128, (g + 1) * 128))
                            pr.append(pg)
                        for g in range(2):
                            gs = slice((4 + g) * 128, (5 + g) * 128)
                            pi = ppool.tile([128, CH], F32, tag="pn", bufs=4,
                                            name=f"pi{t}{c}{layer}{g}")
                            mm_gi(pi[:], gs, True, True)
                            pin.append(pi)
                            pp = ppool.tile([128, CH], F32, tag="pn", bufs=4,
                                            name=f"pp{t}{c}{layer}{g}")
                            mm(pp[:], wh[0][:, gs], hprev[:, 0:CH], True, False)
                            mm(pp[:], wh[1][:, gs], hprev[:, CH:2 * CH], False, True)
                            phn.append(pp)
                        for g in range(2):
                            pg = ppool.tile([128, CH], F32, tag="pz", bufs=2,
                                            name=f"pz{t}{c}{layer}{g}")
                            grp4(pg, slice((2 + g) * 128, (3 + g) * 128))
                            pz.append(pg)

                        # ---- gate chain (ACT + DVE), in dependency order ----
                        rg, zg = [], []
                        for g in range(2):
                            r_ = wpool.tile([128, CH], BF16, tag="r", bufs=3,
                                            name=f"r{t}{c}{layer}{g}")
                            nc.scalar.activation(r_[:], pr[g][:], AF.Sigmoid,
                                                 bias=bias_ap(sigc + g))
                            rg.append(r_)
                        tmps, npres = [], []
                        for g in range(2):
                            tmp = wpool.tile([128, CH], BF16, tag="tmp", bufs=3,
                                             name=f"tm{t}{c}{layer}{g}")
                            if zero_bias:
                                nc.vector.tensor_mul(tmp[:], rg[g][:], phn[g][:])
                            else:
                                nc.vector.scalar_tensor_tensor(
                                    tmp[:], phn[g][:], bias_ap(bhnc[g]),
                                    rg[g][:], OP.add, OP.mult)
                            npre = wpool.tile([128, CH], BF16, tag="npre", bufs=3,
                                              name=f"np{t}{c}{layer}{g}")
                            nc.vector.tensor_add(npre[:], tmp[:], pin[g][:])
                            npres.append(npre)
                        for g in range(2):
                            z_ = wpool.tile([128, CH], BF16, tag="z", bufs=3,
                                            name=f"z{t}{c}{layer}{g}")
                            nc.scalar.activation(z_[:], pz[g][:], AF.Sigmoid,
                                                 bias=bias_ap(sigc + 2 + g))
                            zg.append(z_)
                        ns_ = []
                        for g in range(2):
                            n_ = wpool.tile([128, CH], BF16, tag="n", bufs=3,
                                            name=f"n{t}{c}{layer}{g}")
                            nc.scalar.activation(n_[:], npres[g][:], AF.Tanh,
                                                 bias=bias_ap(tanc[g]))
                            ns_.append(n_)
                        hn = wpool.tile([128, 2 * CH], BF16,
                                        tag=("h0" if layer == 0 else "h1"),
                                        bufs=4, name=f"h{layer}_{t}_{c}")
                        for g in range(2):
                            d_ = wpool.tile([128, CH], BF16, tag="d", bufs=3,
                                            name=f"d{t}{c}{layer}{g}")
                            nc.vector.tensor_sub(d_[:], hprev[:, g * CH:(g + 1) * CH],
                                                 ns_[g][:])
                            e_ = wpool.tile([128, CH], BF16, tag="e", bufs=3,
                                            name=f"e{t}{c}{layer}{g}")
                            nc.vector.tensor_mul(e_[:], zg[g][:], d_[:])
                            nc.vector.tensor_add(hn[:, g * CH:(g + 1) * CH],
                                                 ns_[g][:], e_[:])
                        if layer == 0:
                            h0new[c] = hn
                            h0[c] = hn
                        else:
                            h1[c] = hn

                pending_head = (t, [h1[0], h1[1]])
            if pending_head is not None:
                emit_head(*pending_head)
                pending_head = None

    nc.compile()
    return nc


def _host_prep(z, target_tokens, emb, W_lat, b_lat,
               W_ih0, W_hh0, b_ih0, b_hh0,
               W_ih1, W_hh1, b_ih1, b_hh1,
               W1, b1, W2, b2, n_steps=NSTEPS, fp8_embed=True):
    """Build per-core input maps (all float32)."""
    f = np.float32
    z = np.asarray(z, f)
    tt = np.asarray(target_tokens)
    emb = np.asarray(emb, f)
    W_lat = np.asarray(W_lat, f)

    # teacher-forced input tokens: [START, tgt[:,1], ..., tgt[:,T-2]]
    tokens_in = np.concatenate(
        [np.full((B, 1), START_IDX, dtype=np.int64),
         np.asarray(tt[:, 1:T - 1], np.int64)], axis=1)  # [B, 49]
    tokens_in = tokens_in[:, :n_steps]

    emb2 = (emb @ np.asarray(W_ih0, f).T).astype(f)        # [VOCAB, 768]

    # bias packing: 24 columns
    bias = np.zeros((128, 24), f)
    b_ih0 = np.asarray(b_ih0, f); b_hh0 = np.asarray(b_hh0, f)
    b_ih1 = np.asarray(b_ih1, f); b_hh1 = np.asarray(b_hh1, f)
    sig0 = (b_ih0 + b_hh0)[:512].reshape(4, 128)
    sig1 = (b_ih1 + b_hh1)[:512].reshape(4, 128)
    for j in range(4):
        bias[:, j] = sig0[j]
        bias[:, 6 + j] = sig1[j]
    bias[:, 4] = b_ih0[512:640]; bias[:, 5] = b_ih0[640:768]
    bias[:, 10] = b_ih1[512:640]; bias[:, 11] = b_ih1[640:768]
    b1 = np.asarray(b1, f); b2 = np.asarray(b2, f)
    bias[:, 12] = b1[:128]; bias[:, 13] = b1[128:]
    bias[:, 14] = b2[:128]; bias[:VOCAB - 128, 15] = b2[128:]
    b_lat = np.asarray(b_lat, f)
    for j in range(4):
        bias[:, 16 + j] = b_lat[j * 128:(j + 1) * 128]
    bias[:, 20] = b_hh0[512:640]; bias[:, 21] = b_hh0[640:768]
    bias[:, 22] = b_hh1[512:640]; bias[:, 23] = b_hh1[640:768]

    import ml_dtypes
    bf16 = ml_dtypes.bfloat16
    wpack = np.zeros((128, PACK_COLS), bf16)

    def put(name, arr, rows=128):
        o, w = PACK_OFF[name]
        wpack[:rows, o:o + w] = arr.astype(bf16)

    whh0T = np.asarray(W_hh0, f).T
    wih1T = np.asarray(W_ih1, f).T
    whh1T = np.asarray(W_hh1, f).T
    w1T = np.asarray(W1, f).T
    w2T = np.asarray(W2, f).T
    put("emb2a", emb2[0:128])
    put("emb2b", emb2[128:VOCAB], rows=VOCAB - 128)
    put("whh0k0", whh0T[0:128]); put("whh0k1", whh0T[128:256])
    put("wih1k0", wih1T[0:128]); put("wih1k1", wih1T[128:256])
    put("whh1k0", whh1T[0:128]); put("whh1k1", whh1T[128:256])
    put("w1k0", w1T[0:128]); put("w1k1", w1T[128:256])
    put("w2k0", w2T[0:128]); put("w2k1", w2T[128:256])
    put("wlat", W_lat.T, rows=LATENT)
    put("biases", bias)

    if fp8_embed:
        import ml_dtypes as _md
        fp8 = _md.float8_e4m3
        emb2dr = np.zeros((74, 2, G3), np.float32)
        emb2dr[:, 0, :] = emb2[0::2][:74]
        emb2dr[:, 1, :] = emb2[1::2][:74]
        emb2dr = emb2dr.reshape(74, 2 * G3).astype(fp8)

    in_maps = []
    zo, zw = PACK_OFF["zT"]
    for core in range(NCORES):
        rows = slice(core * BL, (core + 1) * BL)
        tok = tokens_in[rows]                      # [BL, n_steps]
        tsteps = np.arange(n_steps)[None, :].repeat(BL, 0)   # [BL, n_steps]
        bidx = np.arange(BL)[:, None].repeat(n_steps, 1)
        wp = wpack.copy()
        wp[:LATENT, zo:zo + zw] = z[rows].T.astype(bf16)
        m = {"wpack": wp}
        if fp8_embed:
            # oh[t, ki, j*BL + b] = (tok[b,t] == 2*ki + j)
            oh = np.zeros((n_steps, 74, 2, BL), np.float32)
            oh[tsteps.ravel(), (tok // 2).ravel(), (tok % 2).ravel(),
               bidx.ravel()] = 1.0
            m["oh"] = oh.reshape(n_steps, 74, 2 * BL).astype(fp8)
            m["emb2dr"] = emb2dr
        else:
            oh = np.zeros((n_steps, VOCAB, BL), f)
            oh[tsteps.ravel(), tok.ravel(), bidx.ravel()] = 1.0
            m["oh"] = oh.astype(bf16)
        in_maps.append(m)
    return in_maps


class _Runner:
    """Compile once; run many times with device-resident inputs (no
    donation) so repeated calls time the NEFF execution itself."""

    def __init__(self, n_steps=NSTEPS, zero_bias=True, fp8_embed=True):
        import jax
        import numpy as _np
        from jax.sharding import Mesh, PartitionSpec, NamedSharding
        from jax.experimental.shard_map import shard_map
        import concourse.bass2jax as b2j
        import concourse.mybir as mybir

        nc = _build_graph(n_steps, zero_bias=zero_bias, fp8_embed=fp8_embed)
        self.fp8_embed = fp8_embed
        b2j.install_neuronx_cc_hook()
        self.nc = nc
        self.n_steps = n_steps

        partition_name = (nc.partition_id_tensor.name
                          if nc.partition_id_tensor else None)
        in_names, out_names, out_avals, zero_outs = [], [], [], []
        for alloc in nc.m.functions[0].allocations:
            if not isinstance(alloc, mybir.MemoryLocationSet):
                continue
            name = alloc.memorylocations[0].name
            if alloc.kind == "ExternalInput":
                if name != partition_name:
                    in_names.append(name)
            elif alloc.kind == "ExternalOutput":
                shape = list(alloc.tensor_shape)
                out_avals.append(jax.core.ShapedArray(shape, _np.float32))
                out_names.append(name)
                zero_outs.append(_np.zeros(shape, _np.float32))
        self.in_names, self.out_names = list(in_names), out_names
        bind_names = list(in_names) + list(out_names)
        if partition_name is not None:
            bind_names.append(partition_name)

        def _body(*args):
            operands = list(args)
            if partition_name is not None:
                operands.append(b2j.partition_id_tensor())
            outs = b2j._bass_exec_p.bind(
                *operands,
                out_avals=tuple(out_avals),
                in_names=tuple(bind_names),
                out_names=tuple(out_names),
                lowering_input_output_aliases=(),
                sim_require_finite=True,
                sim_require_nnan=True,
                nc=nc,
            )
            return tuple(outs)

        devices = jax.devices()[:NCORES]
        mesh = Mesh(np.asarray(devices), ("core",))
        nin = len(in_names) + len(zero_outs)
        self._fn = jax.jit(shard_map(
            _body, mesh=mesh,
            in_specs=(PartitionSpec("core"),) * nin,
            out_specs=(PartitionSpec("core"),) * len(out_names),
            check_rep=False), keep_unused=True)
        self._sharding = NamedSharding(mesh, PartitionSpec("core"))
        self._jax = jax
        self._zero_outs = zero_outs
        self._placed = None

    def place(self, in_maps):
        """Transfer concatenated per-core inputs to the devices once."""
        jax = self._jax
        concat = []
        for name in self.in_names:
            arr = np.concatenate([m[name] for m in in_maps], axis=0)
            concat.append(jax.device_put(arr, self._sharding))
        for z in self._zero_outs:
            zz = np.zeros((NCORES * z.shape[0], *z.shape[1:]), z.dtype)
            concat.append(jax.device_put(zz, self._sharding))
        self._placed = concat

    def run(self):
        outs = self._fn(*self._placed)
        return outs

    def run_blocked(self):
        outs = self._fn(*self._placed)
        for o in outs:
            o.block_until_ready()
        return outs


def _assemble_logits(out_concat, n_steps):
    """out_concat: [NCORES*n_steps, VOCAB, BL] -> [B, n_steps, VOCAB]."""
    o = np.asarray(out_concat).reshape(NCORES, n_steps, VOCAB, BL)
    # [core, t, v, b] -> [core, b, t, v]
    return o.transpose(0, 3, 1, 2).reshape(B, n_steps, VOCAB)


def kernel(z, target_tokens, emb, W_lat, b_lat,
           W_ih0, W_hh0, b_ih0, b_hh0,
           W_ih1, W_hh1, b_ih1, b_hh1,
           W1, b1, W2, b2, _n_steps=NSTEPS, _runner=None):
    if _runner is None:
        zb = all(np.allclose(np.asarray(b), 0.0) for b in
                 (b_lat, b_ih0, b_hh0, b_ih1, b_hh1, b1, b2))
        _runner = _Runner(_n_steps, zero_bias=zb)
    r = _runner
    in_maps = _host_prep(z, target_tokens, emb, W_lat, b_lat,
                         W_ih0, W_hh0, b_ih0, b_hh0,
                         W_ih1, W_hh1, b_ih1, b_hh1,
                         W1, b1, W2, b2, n_steps=_n_steps,
                         fp8_embed=getattr(r, "fp8_embed", True))
    r.place(in_maps)
    outs = r.run_blocked()
    logits = _assemble_logits(outs[r.out_names.index("out")], _n_steps)
    generated = np.asarray(target_tokens)[:, 1:]
    return logits, generated


# revision 30
# speedup vs baseline: 1.0784x; 1.0784x over previous
"""Trainium2 Bass kernel for nn_AutoregressiveFormulaDecoder.

2-layer GRU decoder with teacher forcing, fused MLP head.
Data-parallel over 8 NeuronCores (1024 batch rows per core).

Device layout: "transposed" — features on SBUF partitions, batch on the
free dimension — so weights are the PE-stationary operand and per-feature
biases are per-partition ACT biases.

Per step t (49 steps), per batch chunk of 512:
  - gi0 comes from a one-hot matmul against emb2 = emb @ W_ih0.T
    (one-hot planes are built on host from the integer tokens).
  - r/z gates: gi and gh matmuls ACCUMULATE in the same PSUM bank, then
    one Sigmoid activation with fused per-partition bias reads PSUM.
  - n gate: i_n and h_n kept in separate PSUM banks; fused DVE
    (h_n + b_hn) * r, + i_n, then Tanh with fused bias.
  - h' = n + z*(h - n) on DVE.
  - Head fused per step: relu(W1 @ h1') then W2 @ ... -> logits tile,
    DMA'd straight to DRAM.

All matmuls run in float32r (full f32 storage, 1 cycle/row on PE for
moving dim >= 256) via AP bitcast — no precision-losing casts.
"""

import numpy as np

VOCAB = 148
START_IDX = 1
LATENT = 32
HID = 256
G3 = 3 * HID  # 768
B = 8192
T = 50
NSTEPS = T - 1  # 49
NCORES = 8
BL = B // NCORES  # 1024 batch rows per core
CH = 512          # batch chunk (one PSUM bank of f32)
NCH = BL // CH    # 2


# packed constant layout: name -> (col offset, col width); all float32 columns
_PACK_SPEC = [
    ("emb2a", G3), ("emb2b", G3),
    ("whh0k0", G3), ("whh0k1", G3),
    ("wih1k0", G3), ("wih1k1", G3),
    ("whh1k0", G3), ("whh1k1", G3),
    ("w1k0", HID), ("w1k1", HID),
    ("w2k0", VOCAB), ("w2k1", VOCAB),
    ("wlat", 2 * HID), ("zT", BL), ("biases", 24),
]
PACK_OFF = {}
_o = 0
for _n, _w in _PACK_SPEC:
    PACK_OFF[_n] = (_o, _w)
    _o += _w
PACK_COLS = _o


def _build_graph(n_steps=NSTEPS, zero_bias=True, fp8_embed=True):
    import concourse.bass as bass
    import concourse.bacc as bacc
    import concourse.mybir as mybir
    import concourse.tile as tile

    F32 = mybir.dt.float32
    BF16 = mybir.dt.bfloat16
    FP8 = mybir.dt.float8e4
    DR = mybir.MatmulPerfMode.DoubleRow
    AF = mybir.ActivationFunctionType
    OP = mybir.AluOpType

    nc = bacc.Bacc()

    if fp8_embed:
        oh_d = nc.declare_dram_parameter("oh", [n_steps, 74, 2 * BL], FP8,
                                         isOutput=False)
        emb2dr_d = nc.declare_dram_parameter("emb2dr", [74, 2 * G3], FP8,
                                             isOutput=False)
    else:
        oh_d = nc.declare_dram_parameter("oh", [n_steps, VOCAB, BL], BF16,
                                         isOutput=False)
    wpack_d = nc.declare_dram_parameter("wpack", [128, PACK_COLS], BF16, isOutput=False)
    out_d = nc.declare_dram_parameter("out", [n_steps, VOCAB, BL], F32, isOutput=True)

    with tile.TileContext(nc) as tc:
        with (
            tc.tile_pool(name="const", bufs=1) as cpool,
            tc.tile_pool(name="io", bufs=4) as iopool,
            tc.tile_pool(name="work", bufs=2) as wpool,
            tc.tile_pool(name="psum", bufs=1, space="PSUM") as ppool,
        ):
            # ---- one DMA for every constant ----
            wpk = cpool.tile([128, PACK_COLS], BF16)
            nc.sync.dma_start(wpk[:], wpack_d[:, :])
            if fp8_embed:
                emb2dr = cpool.tile([74, 2 * G3], FP8)
                nc.sync.dma_start(emb2dr[:], emb2dr_d[:, :])

            def P(name, rows=128):
                o, w = PACK_OFF[name]
                return wpk[0:rows, o:o + w]

            emb2a = P("emb2a")
            emb2b = P("emb2b", rows=VOCAB - 128)
            whh0 = [P("whh0k0"), P("whh0k1")]
            wih1 = [P("wih1k0"), P("wih1k1")]
            whh1 = [P("whh1k0"), P("whh1k1")]
            w1 = [P("w1k0"), P("w1k1")]
            w2 = [P("w2k0"), P("w2k1")]
            wlat = P("wlat", rows=LATENT)
            zT = P("zT", rows=LATENT)

            def bias_ap(col, rows=128):
                o, _ = PACK_OFF["biases"]
                return wpk[0:rows, o + col:o + col + 1]

            def mm(pt, lhsT, rhs, start, stop):
                nc.tensor.matmul(pt, lhsT, rhs, start=start, stop=stop)

            # ---- init hidden state: hT = W_lat @ zT + b_lat ----
            h0 = [None] * NCH   # wide [128, (k,512)] bf16 per chunk
            h1 = [None] * NCH
            for c in range(NCH):
                cs = slice(c * CH, (c + 1) * CH)
                h0[c] = wpool.tile([128, 2 * CH], BF16, tag="h0", bufs=4,
                                   name=f"h0i{c}")
                h1[c] = wpool.tile([128, 2 * CH], BF16, tag="h1", bufs=4,
                                   name=f"h1i{c}")
                for m in range(4):
                    ph = ppool.tile([128, CH], F32, tag="pn", bufs=4,
                                    name=f"pinit{c}_{m}")
                    mm(ph[:], wlat[:, m * 128:(m + 1) * 128], zT[:, cs],
                       True, True)
                    dst = (h0[c] if m < 2 else h1[c])
                    nc.scalar.activation(dst[:, (m % 2) * CH:(m % 2 + 1) * CH],
                                         ph[:], AF.Identity,
                                         bias=bias_ap(16 + m))

            def emit_head(t, h1s):
                for c in range(NCH):
                    cs = slice(c * CH, (c + 1) * CH)
                    hdd = wpool.tile([128, 2 * CH], BF16, tag="hdd", bufs=4,
                                     name=f"hdd{t}_{c}")
                    phds = []
                    for m in range(2):
                        ms = slice(m * 128, (m + 1) * 128)
                        phd = ppool.tile([128, CH], F32, tag="pr", bufs=2,
                                         name=f"phd{t}{c}{m}")
                        mm(phd[:], w1[0][:, ms], h1s[c][:, 0:CH], True, False)
                        mm(phd[:], w1[1][:, ms], h1s[c][:, CH:2 * CH], False, True)
                        phds.append(phd)
                    for m in range(2):
                        nc.scalar.activation(hdd[:, m * CH:(m + 1) * CH],
                                             phds[m][:], AF.Relu,
                                             bias=bias_ap(12 + m))
                    pl0 = ppool.tile([128, CH], F32, tag="pn", bufs=4,
                                     name=f"pl0{t}{c}")
                    mm(pl0[:], w2[0][:, 0:128], hdd[:, 0:CH], True, False)
                    mm(pl0[:], w2[1][:, 0:128], hdd[:, CH:2 * CH], False, True)
                    pl1 = ppool.tile([VOCAB - 128, CH], F32, tag="pn", bufs=4,
                                     name=f"pl1{t}{c}")
                    mm(pl1[:], w2[0][:, 128:VOCAB], hdd[:, 0:CH], True, False)
                    mm(pl1[:], w2[1][:, 128:VOCAB], hdd[:, CH:2 * CH], False, True)
                    lg0 = iopool.tile([128, CH], F32, tag="lg0",
                                      name=f"lg0{t}{c}")
                    lg1 = iopool.tile([VOCAB - 128, CH], F32, tag="lg1",
                                      name=f"lg1{t}{c}")
                    nc.scalar.activation(lg0[:], pl0[:], AF.Identity,
                                         bias=bias_ap(14))
                    nc.scalar.activation(lg1[:], pl1[:], AF.Identity,
                                         bias=bias_ap(15, rows=VOCAB - 128))
                    nc.sync.dma_start(out_d[t, 0:128, cs], lg0[:])
                    nc.sync.dma_start(out_d[t, 128:VOCAB, cs], lg1[:])

            pending_head = None

            # ---- time loop ----
            # Emission order = per-engine execution order. Emit chunk c's
            # matmuls, then its gate chain; chunk c+1's matmuls fill the PE
            # while chunk c's ACT/DVE chain runs. z-gate PSUM groups are
            # emitted last within a chunk (z is needed late) to cut peak
            # PSUM pressure.
            for t in range(n_steps):
                ohs = []
                for c in range(NCH):
                    cs = slice(c * CH, (c + 1) * CH)
                    if fp8_embed:
                        ohc = iopool.tile([74, 2 * CH], FP8, tag="oha",
                                          name=f"oh{t}_{c}")
                        nc.sync.dma_start(
                            ohc[:], oh_d[t].rearrange(
                                "k (j b) -> k j b", j=2)[:, :, cs])
                        ohs.append(ohc)
                    else:
                        oha = iopool.tile([128, CH], BF16, tag="oha",
                                          name=f"oha{t}_{c}")
                        nc.sync.dma_start(oha[:], oh_d[t, 0:128, cs])
                        ohb = iopool.tile([VOCAB - 128, CH], BF16, tag="ohb",
                                          name=f"ohb{t}_{c}")
                        nc.sync.dma_start(ohb[:], oh_d[t, 128:VOCAB, cs])
                        ohs.append((oha, ohb))

                h0new = [None] * NCH
                for layer in range(2):
                    if layer == 1 and pending_head is not None:
                        emit_head(*pending_head)
                        pending_head = None
                    if layer == 0:
                        wh = whh0
                        sigc, tanc, bhnc = 0, (4, 5), (20, 21)
                    else:
                        wh = whh1
                        sigc, tanc, bhnc = 6, (10, 11), (22, 23)

                    for c in range(NCH):
                        hprev = h0[c] if layer == 0 else h1[c]
                        use_dr = fp8_embed and layer == 0
                        if layer == 0:
                            if not fp8_embed:
                                ia, ib = emb2a, emb2b
                                ra, rb = ohs[c]
                            else:
                                oh_rhs = ohs[c].rearrange("k (j b) -> k j b", j=2)
                        else:
                            ia, ib = wih1[0], wih1[1]
                            ra = h0new[c][:, 0:CH]
                            rb = h0new[c][:, CH:2 * CH]

                        def mm_gi(pg, gs, start, stop):
                            # gi contribution for gate rows gs
                            if use_dr:
                                lhs = emb2dr.rearrange(
                                    "k (j m) -> k j m", j=2)[:, :, gs]
                                nc.tensor.matmul(pg, lhs, oh_rhs,
                                                 start=start, stop=stop,
                                                 perf_mode=DR)
                            else:
                                mm(pg, ia[:, gs], ra, start, False)
                                mm(pg, ib[:, gs], rb, False, stop)

                        def grp4(pg, gs):
                            mm(pg[:], wh[0][:, gs], hprev[:, 0:CH], True, False)
                            mm(pg[:], wh[1][:, gs], hprev[:, CH:2 * CH], False, False)
                            mm_gi(pg[:], gs, False, True)

                        # r first, then n-gate psum, z last
                        pr, pin, phn, pz = [], [], [], []
                        for g in range(2):
                            pg = ppool.tile([128, CH], F32, tag="pr", bufs=2,
                                            name=f"pr{t}{c}{layer}{g}")
                            grp4(pg, slice(g * 128, (g + 1) * 128))
                            pr.append(pg)
                        for g in range(2):
                            gs = slice((4 + g) * 128, (5 + g) * 128)
                            pi = ppool.tile([128, CH], F32, tag="pn", bufs=4,
                                            name=f"pi{t}{c}{layer}{g}")
                            mm_gi(pi[:], gs, True, True)
                            pin.append(pi)
                            pp = ppool.tile([128, CH], F32, tag="pn", bufs=4,
                                            name=f"pp{t}{c}{layer}{g}")
                            mm(pp[:], wh[0][:, gs], hprev[:, 0:CH], True, False)
                            mm(pp[:], wh[1][:, gs], hprev[:, CH:2 * CH], False, True)
                            phn.append(pp)
                        for g in range(2):
                            pg = ppool.tile([128, CH], F32, tag="pz", bufs=2,
                                            name=f"pz{t}{c}{layer}{g}")
                            grp4(pg, slice((2 + g) * 128, (3 + g) * 128))
                            pz.append(pg)

                        # ---- gate chain (ACT + DVE), in dependency order ----
                        rg, zg = [], []
                        for g in range(2):
                            r_ = wpool.tile([128, CH], BF16, tag="r", bufs=3,
                                            name=f"r{t}{c}{layer}{g}")
                            nc.scalar.activation(r_[:], pr[g][:], AF.Sigmoid,
                                                 bias=bias_ap(sigc + g))
                            rg.append(r_)
                        tmps, npres = [], []
                        for g in range(2):
                            tmp = wpool.tile([128, CH], BF16, tag="tmp", bufs=3,
                                             name=f"tm{t}{c}{layer}{g}")
                            if zero_bias:
                                nc.vector.tensor_mul(tmp[:], rg[g][:], phn[g][:])
                            else:
                                nc.vector.scalar_tensor_tensor(
                                    tmp[:], phn[g][:], bias_ap(bhnc[g]),
                                    rg[g][:], OP.add, OP.mult)
                            npre = wpool.tile([128, CH], BF16, tag="npre", bufs=3,
                                              name=f"np{t}{c}{layer}{g}")
                            nc.vector.tensor_add(npre[:], tmp[:], pin[g][:])
                            npres.append(npre)
                        for g in range(2):
                            z_ = wpool.tile([128, CH], BF16, tag="z", bufs=3,
                                            name=f"z{t}{c}{layer}{g}")
                            nc.scalar.activation(z_[:], pz[g][:], AF.Sigmoid,
                                                 bias=bias_ap(sigc + 2 + g))
                            zg.append(z_)
                        ns_ = []
                        for g in range(2):
                            n_ = wpool.tile([128, CH], BF16, tag="n", bufs=3,
                                            name=f"n{t}{c}{layer}{g}")
                            nc.scalar.activation(n_[:], npres[g][:], AF.Tanh,
                                                 bias=bias_ap(tanc[g]))
                            ns_.append(n_)
                        hn = wpool.tile([128, 2 * CH], BF16,
                                        tag=("h0" if layer == 0 else "h1"),
                                        bufs=4, name=f"h{layer}_{t}_{c}")
                        for g in range(2):
                            d_ = wpool.tile([128, CH], BF16, tag="d", bufs=3,
                                            name=f"d{t}{c}{layer}{g}")
                            nc.vector.tensor_sub(d_[:], hprev[:, g * CH:(g + 1) * CH],
                                                 ns_[g][:])
                            e_ = wpool.tile([128, CH], BF16, tag="e", bufs=3,
                                            name=f"e{t}{c}{layer}{g}")
                            nc.vector.tensor_mul(e_[:], zg[g][:], d_[:])
                            nc.vector.tensor_add(hn[:, g * CH:(g + 1) * CH],
                                                 ns_[g][:], e_[:])
                        if layer == 0:
                            h0new[c] = hn
                            h0[c] = hn
                        else:
                            h1[c] = hn

                pending_head = (t, [h1[0], h1[1]])
            if pending_head is not None:
                emit_head(*pending_head)
                pending_head = None

    nc.compile()
    return nc


def _host_prep(z, target_tokens, emb, W_lat, b_lat,
               W_ih0, W_hh0, b_ih0, b_hh0,
               W_ih1, W_hh1, b_ih1, b_hh1,
               W1, b1, W2, b2, n_steps=NSTEPS, fp8_embed=True):
    """Build per-core input maps (all float32)."""
    f = np.float32
    z = np.asarray(z, f)
    tt = np.asarray(target_tokens)
    emb = np.asarray(emb, f)
    W_lat = np.asarray(W_lat, f)

    # teacher-forced input tokens: [START, tgt[:,1], ..., tgt[:,T-2]]
    tokens_in = np.concatenate(
        [np.full((B, 1), START_IDX, dtype=np.int64),
         np.asarray(tt[:, 1:T - 1], np.int64)], axis=1)  # [B, 49]
    tokens_in = tokens_in[:, :n_steps]

    emb2 = (emb @ np.asarray(W_ih0, f).T).astype(f)        # [VOCAB, 768]

    # bias packing: 24 columns
    bias = np.zeros((128, 24), f)
    b_ih0 = np.asarray(b_ih0, f); b_hh0 = np.asarray(b_hh0, f)
    b_ih1 = np.asarray(b_ih1, f); b_hh1 = np.asarray(b_hh1, f)
    sig0 = (b_ih0 + b_hh0)[:512].reshape(4, 128)
    sig1 = (b_ih1 + b_hh1)[:512].reshape(4, 128)
    for j in range(4):
        bias[:, j] = sig0[j]
        bias[:, 6 + j] = sig1[j]
    bias[:, 4] = b_ih0[512:640]; bias[:, 5] = b_ih0[640:768]
    bias[:, 10] = b_ih1[512:640]; bias[:, 11] = b_ih1[640:768]
    b1 = np.asarray(b1, f); b2 = np.asarray(b2, f)
    bias[:, 12] = b1[:128]; bias[:, 13] = b1[128:]
    bias[:, 14] = b2[:128]; bias[:VOCAB - 128, 15] = b2[128:]
    b_lat = np.asarray(b_lat, f)
    for j in range(4):
        bias[:, 16 + j] = b_lat[j * 128:(j + 1) * 128]
    bias[:, 20] = b_hh0[512:640]; bias[:, 21] = b_hh0[640:768]
    bias[:, 22] = b_hh1[512:640]; bias[:, 23] = b_hh1[640:768]

    import ml_dtypes
    bf16 = ml_dtypes.bfloat16
    wpack = np.zeros((128, PACK_COLS), bf16)

    def put(name, arr, rows=128):
        o, w = PACK_OFF[name]
        wpack[:rows, o:o + w] = arr.astype(bf16)

    whh0T = np.asarray(W_hh0, f).T
    wih1T = np.asarray(W_ih1, f).T
    whh1T = np.asarray(W_hh1, f).T
    w1T = np.asarray(W1, f).T
    w2T = np.asarray(W2, f).T
    put("emb2a", emb2[0:128])
    put("emb2b", emb2[128:VOCAB], rows=VOCAB - 128)
    put("whh0k0", whh0T[0:128]); put("whh0k1", whh0T[128:256])
    put("wih1k0", wih1T[0:128]); put("wih1k1", wih1T[128:256])
    put("whh1k0", whh1T[0:128]); put("whh1k1", whh1T[128:256])
    put("w1k0", w1T[0:128]); put("w1k1", w1T[128:256])
    put("w2k0", w2T[0:128]); put("w2k1", w2T[128:256])
    put("wlat", W_lat.T, rows=LATENT)
    put("biases", bias)

    if fp8_embed:
        import ml_dtypes as _md
        fp8 = _md.float8_e4m3
        # scale table up, one-hot down by an exact power of two: keeps the
        # product identical while lifting table entries out of fp8 subnormals
        emb2dr = np.zeros((74, 2, G3), np.float32)
        emb2dr[:, 0, :] = emb2[0::2][:74]
        emb2dr[:, 1, :] = emb2[1::2][:74]
        emb2dr = (emb2dr * 64.0).reshape(74, 2 * G3).astype(fp8)

    in_maps = []
    zo, zw = PACK_OFF["zT"]
    for core in range(NCORES):
        rows = slice(core * BL, (core + 1) * BL)
        tok = tokens_in[rows]                      # [BL, n_steps]
        tsteps = np.arange(n_steps)[None, :].repeat(BL, 0)   # [BL, n_steps]
        bidx = np.arange(BL)[:, None].repeat(n_steps, 1)
        wp = wpack.copy()
        wp[:LATENT, zo:zo + zw] = z[rows].T.astype(bf16)
        m = {"wpack": wp}
        if fp8_embed:
            # oh[t, ki, j*BL + b] = (tok[b,t] == 2*ki + j)
            oh = np.zeros((n_steps, 74, 2, BL), np.float32)
            oh[tsteps.ravel(), (tok // 2).ravel(), (tok % 2).ravel(),
               bidx.ravel()] = 1.0 / 64.0
            m["oh"] = oh.reshape(n_steps, 74, 2 * BL).astype(fp8)
            m["emb2dr"] = emb2dr
        else:
            oh = np.zeros((n_steps, VOCAB, BL), f)
            oh[tsteps.ravel(), tok.ravel(), bidx.ravel()] = 1.0
            m["oh"] = oh.astype(bf16)
        in_maps.append(m)
    return in_maps


class _Runner:
    """Compile once; run many times with device-resident inputs (no
    donation) so repeated calls time the NEFF execution itself."""

    def __init__(self, n_steps=NSTEPS, zero_bias=True, fp8_embed=True):
        import jax
        import numpy as _np
        from jax.sharding import Mesh, PartitionSpec, NamedSharding
        from jax.experimental.shard_map import shard_map
        import concourse.bass2jax as b2j
        import concourse.mybir as mybir

        nc = _build_graph(n_steps, zero_bias=zero_bias, fp8_embed=fp8_embed)
        self.fp8_embed = fp8_embed
        b2j.install_neuronx_cc_hook()
        self.nc = nc
        self.n_steps = n_steps

        partition_name = (nc.partition_id_tensor.name
                          if nc.partition_id_tensor else None)
        in_names, out_names, out_avals, zero_outs = [], [], [], []
        for alloc in nc.m.functions[0].allocations:
            if not isinstance(alloc, mybir.MemoryLocationSet):
                continue
            name = alloc.memorylocations[0].name
            if alloc.kind == "ExternalInput":
                if name != partition_name:
                    in_names.append(name)
            elif alloc.kind == "ExternalOutput":
                shape = list(alloc.tensor_shape)
                out_avals.append(jax.core.ShapedArray(shape, _np.float32))
                out_names.append(name)
                zero_outs.append(_np.zeros(shape, _np.float32))
        self.in_names, self.out_names = list(in_names), out_names
        bind_names = list(in_names) + list(out_names)
        if partition_name is not None:
            bind_names.append(partition_name)

        def _body(*args):
            operands = list(args)
            if partition_name is not None:
                operands.append(b2j.partition_id_tensor())
            outs = b2j._bass_exec_p.bind(
                *operands,
                out_avals=tuple(out_avals),
                in_names=tuple(bind_names),
                out_names=tuple(out_names),
                lowering_input_output_aliases=(),
                sim_require_finite=True,
                sim_require_nnan=True,
                nc=nc,
            )
            return tuple(outs)

        devices = jax.devices()[:NCORES]
        mesh = Mesh(np.asarray(devices), ("core",))
        nin = len(in_names) + len(zero_outs)
        self._fn = jax.jit(shard_map(
            _body, mesh=mesh,
            in_specs=(PartitionSpec("core"),) * nin,
            out_specs=(PartitionSpec("core"),) * len(out_names),
            check_rep=False), keep_unused=True)
        self._sharding = NamedSharding(mesh, PartitionSpec("core"))
        self._jax = jax
        self._zero_outs = zero_outs
        self._placed = None

    def place(self, in_maps):
        """Transfer concatenated per-core inputs to the devices once."""
        jax = self._jax
        concat = []
        for name in self.in_names:
            arr = np.concatenate([m[name] for m in in_maps], axis=0)
            concat.append(jax.device_put(arr, self._sharding))
        for z in self._zero_outs:
            zz = np.zeros((NCORES * z.shape[0], *z.shape[1:]), z.dtype)
            concat.append(jax.device_put(zz, self._sharding))
        self._placed = concat

    def run(self):
        outs = self._fn(*self._placed)
        return outs

    def run_blocked(self):
        outs = self._fn(*self._placed)
        for o in outs:
            o.block_until_ready()
        return outs


def _assemble_logits(out_concat, n_steps):
    """out_concat: [NCORES*n_steps, VOCAB, BL] -> [B, n_steps, VOCAB]."""
    o = np.asarray(out_concat).reshape(NCORES, n_steps, VOCAB, BL)
    # [core, t, v, b] -> [core, b, t, v]
    return o.transpose(0, 3, 1, 2).reshape(B, n_steps, VOCAB)


def kernel(z, target_tokens, emb, W_lat, b_lat,
           W_ih0, W_hh0, b_ih0, b_hh0,
           W_ih1, W_hh1, b_ih1, b_hh1,
           W1, b1, W2, b2, _n_steps=NSTEPS, _runner=None):
    if _runner is None:
        zb = all(np.allclose(np.asarray(b), 0.0) for b in
                 (b_lat, b_ih0, b_hh0, b_ih1, b_hh1, b1, b2))
        _runner = _Runner(_n_steps, zero_bias=zb)
    r = _runner
    in_maps = _host_prep(z, target_tokens, emb, W_lat, b_lat,
                         W_ih0, W_hh0, b_ih0, b_hh0,
                         W_ih1, W_hh1, b_ih1, b_hh1,
                         W1, b1, W2, b2, n_steps=_n_steps,
                         fp8_embed=getattr(r, "fp8_embed", True))
    r.place(in_maps)
    outs = r.run_blocked()
    logits = _assemble_logits(outs[r.out_names.index("out")], _n_steps)
    generated = np.asarray(target_tokens)[:, 1:]
    return logits, generated


# revision 33
# speedup vs baseline: 1.0828x; 1.0041x over previous
"""Trainium2 Bass kernel for nn_AutoregressiveFormulaDecoder.

2-layer GRU decoder (HID=256) with teacher forcing + fused MLP head.
Pure data parallel over 8 NeuronCores: 1024 batch rows per core, no
collectives; host shards inputs and reassembles the output.

Device layout is "transposed" (features on SBUF partitions, batch on the
free dim) so weights are the PE-stationary operand and per-feature biases
are per-partition ACT biases.

Per step (49 steps), per 512-wide batch chunk:
  - the input-side projection gi0 = (emb @ W_ih0.T)[token] is computed as
    a one-hot matmul; for the r/z gates it runs in fp8e4m3 with
    perf_mode=DoubleRow (the 148-deep vocab contraction folds to one
    74-partition MM at 0.5 cyc/row; one-hot entries are exact in fp8 and
    the table/one-hot carry a 64 / 1-64 exact power-of-two rescale).
    The tanh-path i_n keeps bf16 one-hot matmuls - fp8 there doubles the
    end-to-end error while r/z fp8 contributes no measurable error.
  - r/z gates: gi and gh matmuls ACCUMULATE in one PSUM bank, then a
    single Sigmoid with fused per-partition bias reads PSUM directly.
  - n gate: i_n / h_n in separate PSUM banks; DVE r*h_n + i_n, Tanh.
  - h' = n + z*(h - n) as all-bf16 SBUF tensor ops (DVE 2x mode).
  - MLP head (relu(W1@h1), W2@...) is fused, software-pipelined one step
    behind the recurrence.

Scheduling notes (these drive the performance):
  - Tile engines execute their instruction streams IN EMISSION ORDER, so
    chunk c+1's matmuls are emitted between chunk c's matmuls and the
    ops that consume them - the PE stays busy while ACT/DVE run chunk
    c's gate chain.
  - PSUM tiles are tagged by drain class (pr=2, pn=4, pz=2 banks) so a
    new group's bank-reuse waits on an ACT/DVE op that actually fires
    early, not an arbitrary late one.
  - All constants ship in ONE packed bf16 DRAM tensor (single DMA):
    walrus allows at most 1 sync-wait per instruction, so fan-in from
    many DMA queues must be avoided.

Cost-model (CoreSim) predicted exec: ~1.11 ms for the full 49 steps
(PE-busy floor for this decomposition is ~1.0 ms), rel err ~6.4e-3 vs
the float32 reference.
"""

import numpy as np

VOCAB = 148
START_IDX = 1
LATENT = 32
HID = 256
G3 = 3 * HID  # 768
B = 8192
T = 50
NSTEPS = T - 1  # 49
NCORES = 8
BL = B // NCORES  # 1024 batch rows per core
CH = 512          # batch chunk (one PSUM bank of f32)
NCH = BL // CH    # 2


# packed constant layout: name -> (col offset, col width); all float32 columns
_PACK_SPEC = [
    ("emb2a", G3), ("emb2b", G3),
    ("whh0k0", G3), ("whh0k1", G3),
    ("wih1k0", G3), ("wih1k1", G3),
    ("whh1k0", G3), ("whh1k1", G3),
    ("w1k0", HID), ("w1k1", HID),
    ("w2k0", VOCAB), ("w2k1", VOCAB),
    ("wlat", 2 * HID), ("zT", BL), ("biases", 24),
]
PACK_OFF = {}
_o = 0
for _n, _w in _PACK_SPEC:
    PACK_OFF[_n] = (_o, _w)
    _o += _w
PACK_COLS = _o


def _build_graph(n_steps=NSTEPS, zero_bias=True, fp8_embed=True, hybrid_n=True):
    import concourse.bass as bass
    import concourse.bacc as bacc
    import concourse.mybir as mybir
    import concourse.tile as tile

    F32 = mybir.dt.float32
    BF16 = mybir.dt.bfloat16
    FP8 = mybir.dt.float8e4
    DR = mybir.MatmulPerfMode.DoubleRow
    AF = mybir.ActivationFunctionType
    OP = mybir.AluOpType

    nc = bacc.Bacc()

    if fp8_embed:
        oh_d = nc.declare_dram_parameter("oh", [n_steps, 74, 2 * BL], FP8,
                                         isOutput=False)
        emb2dr_d = nc.declare_dram_parameter("emb2dr", [74, 2 * G3], FP8,
                                             isOutput=False)
        if hybrid_n:
            ohbf_d = nc.declare_dram_parameter("ohbf", [n_steps, VOCAB, BL],
                                               BF16, isOutput=False)
    else:
        oh_d = nc.declare_dram_parameter("oh", [n_steps, VOCAB, BL], BF16,
                                         isOutput=False)
    wpack_d = nc.declare_dram_parameter("wpack", [128, PACK_COLS], BF16, isOutput=False)
    out_d = nc.declare_dram_parameter("out", [n_steps, VOCAB, BL], F32, isOutput=True)

    with tile.TileContext(nc) as tc:
        with (
            tc.tile_pool(name="const", bufs=1) as cpool,
            tc.tile_pool(name="io", bufs=4) as iopool,
            tc.tile_pool(name="work", bufs=2) as wpool,
            tc.tile_pool(name="psum", bufs=1, space="PSUM") as ppool,
        ):
            # ---- one DMA for every constant ----
            wpk = cpool.tile([128, PACK_COLS], BF16)
            nc.sync.dma_start(wpk[:], wpack_d[:, :])
            if fp8_embed:
                emb2dr = cpool.tile([74, 2 * G3], FP8)
                nc.sync.dma_start(emb2dr[:], emb2dr_d[:, :])

            def P(name, rows=128):
                o, w = PACK_OFF[name]
                return wpk[0:rows, o:o + w]

            emb2a = P("emb2a")
            emb2b = P("emb2b", rows=VOCAB - 128)
            whh0 = [P("whh0k0"), P("whh0k1")]
            wih1 = [P("wih1k0"), P("wih1k1")]
            whh1 = [P("whh1k0"), P("whh1k1")]
            w1 = [P("w1k0"), P("w1k1")]
            w2 = [P("w2k0"), P("w2k1")]
            wlat = P("wlat", rows=LATENT)
            zT = P("zT", rows=LATENT)

            def bias_ap(col, rows=128):
                o, _ = PACK_OFF["biases"]
                return wpk[0:rows, o + col:o + col + 1]

            def mm(pt, lhsT, rhs, start, stop):
                nc.tensor.matmul(pt, lhsT, rhs, start=start, stop=stop)

            # ---- init hidden state: hT = W_lat @ zT + b_lat ----
            h0 = [None] * NCH   # wide [128, (k,512)] bf16 per chunk
            h1 = [None] * NCH
            for c in range(NCH):
                cs = slice(c * CH, (c + 1) * CH)
                h0[c] = wpool.tile([128, 2 * CH], BF16, tag="h0", bufs=4,
                                   name=f"h0i{c}")
                h1[c] = wpool.tile([128, 2 * CH], BF16, tag="h1", bufs=4,
                                   name=f"h1i{c}")
                for m in range(4):
                    ph = ppool.tile([128, CH], F32, tag="pn", bufs=4,
                                    name=f"pinit{c}_{m}")
                    mm(ph[:], wlat[:, m * 128:(m + 1) * 128], zT[:, cs],
                       True, True)
                    dst = (h0[c] if m < 2 else h1[c])
                    nc.scalar.activation(dst[:, (m % 2) * CH:(m % 2 + 1) * CH],
                                         ph[:], AF.Identity,
                                         bias=bias_ap(16 + m))

            def emit_head(t, h1s):
                for c in range(NCH):
                    cs = slice(c * CH, (c + 1) * CH)
                    hdd = wpool.tile([128, 2 * CH], BF16, tag="hdd", bufs=4,
                                     name=f"hdd{t}_{c}")
                    phds = []
                    for m in range(2):
                        ms = slice(m * 128, (m + 1) * 128)
                        phd = ppool.tile([128, CH], F32, tag="pr", bufs=2,
                                         name=f"phd{t}{c}{m}")
                        mm(phd[:], w1[0][:, ms], h1s[c][:, 0:CH], True, False)
                        mm(phd[:], w1[1][:, ms], h1s[c][:, CH:2 * CH], False, True)
                        phds.append(phd)
                    for m in range(2):
                        nc.scalar.activation(hdd[:, m * CH:(m + 1) * CH],
                                             phds[m][:], AF.Relu,
                                             bias=bias_ap(12 + m))
                    pl0 = ppool.tile([128, CH], F32, tag="pn", bufs=4,
                                     name=f"pl0{t}{c}")
                    mm(pl0[:], w2[0][:, 0:128], hdd[:, 0:CH], True, False)
                    mm(pl0[:], w2[1][:, 0:128], hdd[:, CH:2 * CH], False, True)
                    pl1 = ppool.tile([VOCAB - 128, CH], F32, tag="pn", bufs=4,
                                     name=f"pl1{t}{c}")
                    mm(pl1[:], w2[0][:, 128:VOCAB], hdd[:, 0:CH], True, False)
                    mm(pl1[:], w2[1][:, 128:VOCAB], hdd[:, CH:2 * CH], False, True)
                    lg0 = iopool.tile([128, CH], F32, tag="lg0",
                                      name=f"lg0{t}{c}")
                    lg1 = iopool.tile([VOCAB - 128, CH], F32, tag="lg1",
                                      name=f"lg1{t}{c}")
                    nc.scalar.activation(lg0[:], pl0[:], AF.Identity,
                                         bias=bias_ap(14))
                    nc.scalar.activation(lg1[:], pl1[:], AF.Identity,
                                         bias=bias_ap(15, rows=VOCAB - 128))
                    nc.sync.dma_start(out_d[t, 0:128, cs], lg0[:])
                    nc.sync.dma_start(out_d[t, 128:VOCAB, cs], lg1[:])

            pending_head = None

            # ---- time loop ----
            # Emission order = per-engine execution order. Emit chunk c's
            # matmuls, then its gate chain; chunk c+1's matmuls fill the PE
            # while chunk c's ACT/DVE chain runs. z-gate PSUM groups are
            # emitted last within a chunk (z is needed late) to cut peak
            # PSUM pressure.
            for t in range(n_steps):
                ohs = []
                for c in range(NCH):
                    cs = slice(c * CH, (c + 1) * CH)
                    if fp8_embed:
                        ohc = iopool.tile([74, 2 * CH], FP8, tag="oha",
                                          name=f"oh{t}_{c}")
                        nc.sync.dma_start(
                            ohc[:], oh_d[t].rearrange(
                                "k (j b) -> k j b", j=2)[:, :, cs])
                        if hybrid_n:
                            ohA = iopool.tile([128, CH], BF16, tag="ohA",
                                              name=f"ohA{t}_{c}")
                            nc.sync.dma_start(ohA[:], ohbf_d[t, 0:128, cs])
                            ohB = iopool.tile([VOCAB - 128, CH], BF16, tag="ohB",
                                              name=f"ohB{t}_{c}")
                            nc.sync.dma_start(ohB[:], ohbf_d[t, 128:VOCAB, cs])
                            ohs.append((ohc, (ohA, ohB)))
                        else:
                            ohs.append((ohc, None))
                    else:
                        oha = iopool.tile([128, CH], BF16, tag="oha",
                                          name=f"oha{t}_{c}")
                        nc.sync.dma_start(oha[:], oh_d[t, 0:128, cs])
                        ohb = iopool.tile([VOCAB - 128, CH], BF16, tag="ohb",
                                          name=f"ohb{t}_{c}")
                        nc.sync.dma_start(ohb[:], oh_d[t, 128:VOCAB, cs])
                        ohs.append((oha, ohb))

                h0new = [None] * NCH
                for layer in range(2):
                    if layer == 1 and pending_head is not None:
                        emit_head(*pending_head)
                        pending_head = None
                    if layer == 0:
                        wh = whh0
                        sigc, tanc, bhnc = 0, (4, 5), (20, 21)
                    else:
                        wh = whh1
                        sigc, tanc, bhnc = 6, (10, 11), (22, 23)

                    for c in range(NCH):
                        hprev = h0[c] if layer == 0 else h1[c]
                        use_dr = fp8_embed and layer == 0
                        if layer == 0:
                            if not fp8_embed:
                                ia, ib = emb2a, emb2b
                                ra, rb = ohs[c]
                            else:
                                oh_rhs = ohs[c][0].rearrange("k (j b) -> k j b", j=2)
                                if hybrid_n:
                                    ia, ib = emb2a, emb2b
                                    ra, rb = ohs[c][1]
                        else:
                            ia, ib = wih1[0], wih1[1]
                            ra = h0new[c][:, 0:CH]
                            rb = h0new[c][:, CH:2 * CH]

                        def mm_gi(pg, gs, start, stop, dr=True):
                            # gi contribution for gate rows gs
                            if use_dr and dr:
                                lhs = emb2dr.rearrange(
                                    "k (j m) -> k j m", j=2)[:, :, gs]
                                nc.tensor.matmul(pg, lhs, oh_rhs,
                                                 start=start, stop=stop,
                                                 perf_mode=DR)
                            else:
                                mm(pg, ia[:, gs], ra, start, False)
                                mm(pg, ib[:, gs], rb, False, stop)

                        def grp4(pg, gs):
                            mm(pg[:], wh[0][:, gs], hprev[:, 0:CH], True, False)
                            mm(pg[:], wh[1][:, gs], hprev[:, CH:2 * CH], False, False)
                            mm_gi(pg[:], gs, False, True)

                        # r first, then n-gate psum, z last
                        pr, pin, phn, pz = [], [], [], []
                        for g in range(2):
                            pg = ppool.tile([128, CH], F32, tag="pr", bufs=2,
                                            name=f"pr{t}{c}{layer}{g}")
                            grp4(pg, slice(g * 128, (g + 1) * 128))
                            pr.append(pg)
                        for g in range(2):
                            gs = slice((4 + g) * 128, (5 + g) * 128)
                            pi = ppool.tile([128, CH], F32, tag="pn", bufs=4,
                                            name=f"pi{t}{c}{layer}{g}")
                            mm_gi(pi[:], gs, True, True,
                                  dr=not (fp8_embed and hybrid_n))
                            pin.append(pi)
                            pp = ppool.tile([128, CH], F32, tag="pn", bufs=4,
                                            name=f"pp{t}{c}{layer}{g}")
                            mm(pp[:], wh[0][:, gs], hprev[:, 0:CH], True, False)
                            mm(pp[:], wh[1][:, gs], hprev[:, CH:2 * CH], False, True)
                            phn.append(pp)
                        for g in range(2):
                            pg = ppool.tile([128, CH], F32, tag="pz", bufs=2,
                                            name=f"pz{t}{c}{layer}{g}")
                            grp4(pg, slice((2 + g) * 128, (3 + g) * 128))
                            pz.append(pg)

                        # ---- gate chain (ACT + DVE), in dependency order ----
                        rg, zg = [], []
                        for g in range(2):
                            r_ = wpool.tile([128, CH], BF16, tag="r", bufs=3,
                                            name=f"r{t}{c}{layer}{g}")
                            nc.scalar.activation(r_[:], pr[g][:], AF.Sigmoid,
                                                 bias=bias_ap(sigc + g))
                            rg.append(r_)
                        tmps, npres = [], []
                        for g in range(2):
                            tmp = wpool.tile([128, CH], BF16, tag="tmp", bufs=3,
                                             name=f"tm{t}{c}{layer}{g}")
                            if zero_bias:
                                nc.vector.tensor_mul(tmp[:], rg[g][:], phn[g][:])
                            else:
                                nc.vector.scalar_tensor_tensor(
                                    tmp[:], phn[g][:], bias_ap(bhnc[g]),
                                    rg[g][:], OP.add, OP.mult)
                            npre = wpool.tile([128, CH], BF16, tag="npre", bufs=3,
                                              name=f"np{t}{c}{layer}{g}")
                            nc.vector.tensor_add(npre[:], tmp[:], pin[g][:])
                            npres.append(npre)
                        for g in range(2):
                            z_ = wpool.tile([128, CH], BF16, tag="z", bufs=3,
                                            name=f"z{t}{c}{layer}{g}")
                            nc.scalar.activation(z_[:], pz[g][:], AF.Sigmoid,
                                                 bias=bias_ap(sigc + 2 + g))
                            zg.append(z_)
                        ns_ = []
                        for g in range(2):
                            n_ = wpool.tile([128, CH], BF16, tag="n", bufs=3,
                                            name=f"n{t}{c}{layer}{g}")
                            nc.scalar.activation(n_[:], npres[g][:], AF.Tanh,
                                                 bias=bias_ap(tanc[g]))
                            ns_.append(n_)
                        hn = wpool.tile([128, 2 * CH], BF16,
                                        tag=("h0" if layer == 0 else "h1"),
                                        bufs=4, name=f"h{layer}_{t}_{c}")
                        for g in range(2):
                            d_ = wpool.tile([128, CH], BF16, tag="d", bufs=3,
                                            name=f"d{t}{c}{layer}{g}")
                            nc.vector.tensor_sub(d_[:], hprev[:, g * CH:(g + 1) * CH],
                                                 ns_[g][:])
                            e_ = wpool.tile([128, CH], BF16, tag="e", bufs=3,
                                            name=f"e{t}{c}{layer}{g}")
                            nc.vector.tensor_mul(e_[:], zg[g][:], d_[:])
                            nc.vector.tensor_add(hn[:, g * CH:(g + 1) * CH],
                                                 ns_[g][:], e_[:])
                        if layer == 0:
                            h0new[c] = hn
                            h0[c] = hn
                        else:
                            h1[c] = hn

                pending_head = (t, [h1[0], h1[1]])
            if pending_head is not None:
                emit_head(*pending_head)
                pending_head = None

    nc.compile()
    return nc


def _host_prep(z, target_tokens, emb, W_lat, b_lat,
               W_ih0, W_hh0, b_ih0, b_hh0,
               W_ih1, W_hh1, b_ih1, b_hh1,
               W1, b1, W2, b2, n_steps=NSTEPS, fp8_embed=True, hybrid_n=True):
    """Build per-core input maps (all float32)."""
    f = np.float32
    z = np.asarray(z, f)
    tt = np.asarray(target_tokens)
    emb = np.asarray(emb, f)
    W_lat = np.asarray(W_lat, f)

    # teacher-forced input tokens: [START, tgt[:,1], ..., tgt[:,T-2]]
    tokens_in = np.concatenate(
        [np.full((B, 1), START_IDX, dtype=np.int64),
         np.asarray(tt[:, 1:T - 1], np.int64)], axis=1)  # [B, 49]
    tokens_in = tokens_in[:, :n_steps]

    emb2 = (emb @ np.asarray(W_ih0, f).T).astype(f)        # [VOCAB, 768]

    # bias packing: 24 columns
    bias = np.zeros((128, 24), f)
    b_ih0 = np.asarray(b_ih0, f); b_hh0 = np.asarray(b_hh0, f)
    b_ih1 = np.asarray(b_ih1, f); b_hh1 = np.asarray(b_hh1, f)
    sig0 = (b_ih0 + b_hh0)[:512].reshape(4, 128)
    sig1 = (b_ih1 + b_hh1)[:512].reshape(4, 128)
    for j in range(4):
        bias[:, j] = sig0[j]
        bias[:, 6 + j] = sig1[j]
    bias[:, 4] = b_ih0[512:640]; bias[:, 5] = b_ih0[640:768]
    bias[:, 10] = b_ih1[512:640]; bias[:, 11] = b_ih1[640:768]
    b1 = np.asarray(b1, f); b2 = np.asarray(b2, f)
    bias[:, 12] = b1[:128]; bias[:, 13] = b1[128:]
    bias[:, 14] = b2[:128]; bias[:VOCAB - 128, 15] = b2[128:]
    b_lat = np.asarray(b_lat, f)
    for j in range(4):
        bias[:, 16 + j] = b_lat[j * 128:(j + 1) * 128]
    bias[:, 20] = b_hh0[512:640]; bias[:, 21] = b_hh0[640:768]
    bias[:, 22] = b_hh1[512:640]; bias[:, 23] = b_hh1[640:768]

    import ml_dtypes
    bf16 = ml_dtypes.bfloat16
    wpack = np.zeros((128, PACK_COLS), bf16)

    def put(name, arr, rows=128):
        o, w = PACK_OFF[name]
        wpack[:rows, o:o + w] = arr.astype(bf16)

    whh0T = np.asarray(W_hh0, f).T
    wih1T = np.asarray(W_ih1, f).T
    whh1T = np.asarray(W_hh1, f).T
    w1T = np.asarray(W1, f).T
    w2T = np.asarray(W2, f).T
    put("emb2a", emb2[0:128])
    put("emb2b", emb2[128:VOCAB], rows=VOCAB - 128)
    put("whh0k0", whh0T[0:128]); put("whh0k1", whh0T[128:256])
    put("wih1k0", wih1T[0:128]); put("wih1k1", wih1T[128:256])
    put("whh1k0", whh1T[0:128]); put("whh1k1", whh1T[128:256])
    put("w1k0", w1T[0:128]); put("w1k1", w1T[128:256])
    put("w2k0", w2T[0:128]); put("w2k1", w2T[128:256])
    put("wlat", W_lat.T, rows=LATENT)
    put("biases", bias)

    if fp8_embed:
        import ml_dtypes as _md
        fp8 = _md.float8_e4m3
        # scale table up, one-hot down by an exact power of two: keeps the
        # product identical while lifting table entries out of fp8 subnormals
        emb2dr = np.zeros((74, 2, G3), np.float32)
        emb2dr[:, 0, :] = emb2[0::2][:74]
        emb2dr[:, 1, :] = emb2[1::2][:74]
        emb2dr = (emb2dr * 64.0).reshape(74, 2 * G3).astype(fp8)

    in_maps = []
    zo, zw = PACK_OFF["zT"]
    for core in range(NCORES):
        rows = slice(core * BL, (core + 1) * BL)
        tok = tokens_in[rows]                      # [BL, n_steps]
        tsteps = np.arange(n_steps)[None, :].repeat(BL, 0)   # [BL, n_steps]
        bidx = np.arange(BL)[:, None].repeat(n_steps, 1)
        wp = wpack.copy()
        wp[:LATENT, zo:zo + zw] = z[rows].T.astype(bf16)
        m = {"wpack": wp}
        if fp8_embed:
            # oh[t, ki, j*BL + b] = (tok[b,t] == 2*ki + j)
            oh = np.zeros((n_steps, 74, 2, BL), np.float32)
            oh[tsteps.ravel(), (tok // 2).ravel(), (tok % 2).ravel(),
               bidx.ravel()] = 1.0 / 64.0
            m["oh"] = oh.reshape(n_steps, 74, 2 * BL).astype(fp8)
            m["emb2dr"] = emb2dr
            if hybrid_n:
                ohb_ = np.zeros((n_steps, VOCAB, BL), f)
                ohb_[tsteps.ravel(), tok.ravel(), bidx.ravel()] = 1.0
                m["ohbf"] = ohb_.astype(bf16)
        else:
            oh = np.zeros((n_steps, VOCAB, BL), f)
            oh[tsteps.ravel(), tok.ravel(), bidx.ravel()] = 1.0
            m["oh"] = oh.astype(bf16)
        in_maps.append(m)
    return in_maps


class _Runner:
    """Compile once; run many times with device-resident inputs (no
    donation) so repeated calls time the NEFF execution itself."""

    def __init__(self, n_steps=NSTEPS, zero_bias=True, fp8_embed=True, hybrid_n=True):
        import jax
        import numpy as _np
        from jax.sharding import Mesh, PartitionSpec, NamedSharding
        from jax.experimental.shard_map import shard_map
        import concourse.bass2jax as b2j
        import concourse.mybir as mybir

        nc = _build_graph(n_steps, zero_bias=zero_bias, fp8_embed=fp8_embed,
                          hybrid_n=hybrid_n)
        self.fp8_embed = fp8_embed
        self.hybrid_n = hybrid_n
        b2j.install_neuronx_cc_hook()
        self.nc = nc
        self.n_steps = n_steps

        partition_name = (nc.partition_id_tensor.name
                          if nc.partition_id_tensor else None)
        in_names, out_names, out_avals, zero_outs = [], [], [], []
        for alloc in nc.m.functions[0].allocations:
            if not isinstance(alloc, mybir.MemoryLocationSet):
                continue
            name = alloc.memorylocations[0].name
            if alloc.kind == "ExternalInput":
                if name != partition_name:
                    in_names.append(name)
            elif alloc.kind == "ExternalOutput":
                shape = list(alloc.tensor_shape)
                out_avals.append(jax.core.ShapedArray(shape, _np.float32))
                out_names.append(name)
                zero_outs.append(_np.zeros(shape, _np.float32))
        self.in_names, self.out_names = list(in_names), out_names
        bind_names = list(in_names) + list(out_names)
        if partition_name is not None:
            bind_names.append(partition_name)

        def _body(*args):
            operands = list(args)
            if partition_name is not None:
                operands.append(b2j.partition_id_tensor())
            outs = b2j._bass_exec_p.bind(
                *operands,
                out_avals=tuple(out_avals),
                in_names=tuple(bind_names),
                out_names=tuple(out_names),
                lowering_input_output_aliases=(),
                sim_require_finite=True,
                sim_require_nnan=True,
                nc=nc,
            )
            return tuple(outs)

        devices = jax.devices()[:NCORES]
        mesh = Mesh(np.asarray(devices), ("core",))
        nin = len(in_names) + len(zero_outs)
        self._fn = jax.jit(shard_map(
            _body, mesh=mesh,
            in_specs=(PartitionSpec("core"),) * nin,
            out_specs=(PartitionSpec("core"),) * len(out_names),
            check_rep=False), keep_unused=True)
        self._sharding = NamedSharding(mesh, PartitionSpec("core"))
        self._jax = jax
        self._zero_outs = zero_outs
        self._placed = None

    def place(self, in_maps):
        """Transfer concatenated per-core inputs to the devices once."""
        jax = self._jax
        concat = []
        for name in self.in_names:
            arr = np.concatenate([m[name] for m in in_maps], axis=0)
            concat.append(jax.device_put(arr, self._sharding))
        for z in self._zero_outs:
            zz = np.zeros((NCORES * z.shape[0], *z.shape[1:]), z.dtype)
            concat.append(jax.device_put(zz, self._sharding))
        self._placed = concat

    def run(self):
        outs = self._fn(*self._placed)
        return outs

    def run_blocked(self):
        outs = self._fn(*self._placed)
        for o in outs:
            o.block_until_ready()
        return outs


def _assemble_logits(out_concat, n_steps):
    """out_concat: [NCORES*n_steps, VOCAB, BL] -> [B, n_steps, VOCAB]."""
    o = np.asarray(out_concat).reshape(NCORES, n_steps, VOCAB, BL)
    # [core, t, v, b] -> [core, b, t, v]
    return o.transpose(0, 3, 1, 2).reshape(B, n_steps, VOCAB)


def kernel(z, target_tokens, emb, W_lat, b_lat,
           W_ih0, W_hh0, b_ih0, b_hh0,
           W_ih1, W_hh1, b_ih1, b_hh1,
           W1, b1, W2, b2, _n_steps=NSTEPS, _runner=None):
    if _runner is None:
        zb = all(np.allclose(np.asarray(b), 0.0) for b in
                 (b_lat, b_ih0, b_hh0, b_ih1, b_hh1, b1, b2))
        _runner = _Runner(_n_steps, zero_bias=zb)
    r = _runner
    in_maps = _host_prep(z, target_tokens, emb, W_lat, b_lat,
                         W_ih0, W_hh0, b_ih0, b_hh0,
                         W_ih1, W_hh1, b_ih1, b_hh1,
                         W1, b1, W2, b2, n_steps=_n_steps,
                         fp8_embed=getattr(r, "fp8_embed", True),
                         hybrid_n=getattr(r, "hybrid_n", True))
    r.place(in_maps)
    outs = r.run_blocked()
    logits = _assemble_logits(outs[r.out_names.index("out")], _n_steps)
    generated = np.asarray(target_tokens)[:, 1:]
    return logits, generated


# revision 34
# speedup vs baseline: 99.6607x; 92.0409x over previous
"""Trainium2 Bass kernel for nn_AutoregressiveFormulaDecoder.

2-layer GRU decoder (HID=256) with teacher forcing + fused MLP head.
Pure data parallel over 8 NeuronCores: 1024 batch rows per core, no
collectives; host shards inputs and reassembles the output.

Device layout is "transposed" (features on SBUF partitions, batch on the
free dim) so weights are the PE-stationary operand and per-feature biases
are per-partition ACT biases.

Per step (49 steps), per 512-wide batch chunk:
  - the input-side projection gi0 = (emb @ W_ih0.T)[token] is computed as
    a one-hot matmul; for the r/z gates it runs in fp8e4m3 with
    perf_mode=DoubleRow (the 148-deep vocab contraction folds to one
    74-partition MM at 0.5 cyc/row; one-hot entries are exact in fp8 and
    the table/one-hot carry a 64 / 1-64 exact power-of-two rescale).
    The tanh-path i_n keeps bf16 one-hot matmuls - fp8 there doubles the
    end-to-end error while r/z fp8 contributes no measurable error.
  - r/z gates: gi and gh matmuls ACCUMULATE in one PSUM bank, then a
    single Sigmoid with fused per-partition bias reads PSUM directly.
  - n gate: i_n / h_n in separate PSUM banks; DVE r*h_n + i_n, Tanh.
  - h' = n + z*(h - n) as all-bf16 SBUF tensor ops (DVE 2x mode).
  - MLP head (relu(W1@h1), W2@...) is fused, software-pipelined one step
    behind the recurrence.

Scheduling notes (these drive the performance):
  - Tile engines execute their instruction streams IN EMISSION ORDER, so
    chunk c+1's matmuls are emitted between chunk c's matmuls and the
    ops that consume them - the PE stays busy while ACT/DVE run chunk
    c's gate chain.
  - PSUM tiles are tagged by drain class (pr=2, pn=4, pz=2 banks) so a
    new group's bank-reuse waits on an ACT/DVE op that actually fires
    early, not an arbitrary late one.
  - All constants ship in ONE packed bf16 DRAM tensor (single DMA):
    walrus allows at most 1 sync-wait per instruction, so fan-in from
    many DMA queues must be avoided.

Cost-model (CoreSim) predicted exec: ~1.11 ms for the full 49 steps
(PE-busy floor for this decomposition is ~1.0 ms), rel err ~6.4e-3 vs
the float32 reference.
"""

import numpy as np

VOCAB = 148
START_IDX = 1
LATENT = 32
HID = 256
G3 = 3 * HID  # 768
B = 8192
T = 50
NSTEPS = T - 1  # 49
NCORES = 8
BL = B // NCORES  # 1024 batch rows per core
CH = 512          # batch chunk (one PSUM bank of f32)
NCH = BL // CH    # 2


# packed constant layout: name -> (col offset, col width); all float32 columns
_PACK_SPEC = [
    ("emb2a", G3), ("emb2b", G3),
    ("whh0k0", G3), ("whh0k1", G3),
    ("wih1k0", G3), ("wih1k1", G3),
    ("whh1k0", G3), ("whh1k1", G3),
    ("w1k0", HID), ("w1k1", HID),
    ("w2k0", VOCAB), ("w2k1", VOCAB),
    ("wlat", 2 * HID), ("zT", BL), ("biases", 24),
]
PACK_OFF = {}
_o = 0
for _n, _w in _PACK_SPEC:
    PACK_OFF[_n] = (_o, _w)
    _o += _w
PACK_COLS = _o


def _build_graph(n_steps=NSTEPS, zero_bias=True, fp8_embed=True, hybrid_n=True):
    import concourse.bass as bass
    import concourse.bacc as bacc
    import concourse.mybir as mybir
    import concourse.tile as tile

    F32 = mybir.dt.float32
    BF16 = mybir.dt.bfloat16
    FP8 = mybir.dt.float8e4
    DR = mybir.MatmulPerfMode.DoubleRow
    AF = mybir.ActivationFunctionType
    OP = mybir.AluOpType

    nc = bacc.Bacc()

    if fp8_embed:
        oh_d = nc.declare_dram_parameter("oh", [n_steps, 74, 2 * BL], FP8,
                                         isOutput=False)
        emb2dr_d = nc.declare_dram_parameter("emb2dr", [74, 2 * G3], FP8,
                                             isOutput=False)
        if hybrid_n:
            ohbf_d = nc.declare_dram_parameter("ohbf", [n_steps, VOCAB, BL],
                                               BF16, isOutput=False)
    else:
        oh_d = nc.declare_dram_parameter("oh", [n_steps, VOCAB, BL], BF16,
                                         isOutput=False)
    wpack_d = nc.declare_dram_parameter("wpack", [128, PACK_COLS], BF16, isOutput=False)
    out_d = nc.declare_dram_parameter("out", [n_steps, VOCAB, BL], F32, isOutput=True)

    with tile.TileContext(nc) as tc:
        with (
            tc.tile_pool(name="const", bufs=1) as cpool,
            tc.tile_pool(name="io", bufs=4) as iopool,
            tc.tile_pool(name="work", bufs=2) as wpool,
            tc.tile_pool(name="psum", bufs=1, space="PSUM") as ppool,
        ):
            # ---- one DMA for every constant ----
            wpk = cpool.tile([128, PACK_COLS], BF16)
            nc.sync.dma_start(wpk[:], wpack_d[:, :])
            if fp8_embed:
                emb2dr = cpool.tile([74, 2 * G3], FP8)
                nc.sync.dma_start(emb2dr[:], emb2dr_d[:, :])

            def P(name, rows=128):
                o, w = PACK_OFF[name]
                return wpk[0:rows, o:o + w]

            emb2a = P("emb2a")
            emb2b = P("emb2b", rows=VOCAB - 128)
            whh0 = [P("whh0k0"), P("whh0k1")]
            wih1 = [P("wih1k0"), P("wih1k1")]
            whh1 = [P("whh1k0"), P("whh1k1")]
            w1 = [P("w1k0"), P("w1k1")]
            w2 = [P("w2k0"), P("w2k1")]
            wlat = P("wlat", rows=LATENT)
            zT = P("zT", rows=LATENT)

            def bias_ap(col, rows=128):
                o, _ = PACK_OFF["biases"]
                return wpk[0:rows, o + col:o + col + 1]

            def mm(pt, lhsT, rhs, start, stop):
                nc.tensor.matmul(pt, lhsT, rhs, start=start, stop=stop)

            # ---- init hidden state: hT = W_lat @ zT + b_lat ----
            h0 = [None] * NCH   # wide [128, (k,512)] bf16 per chunk
            h1 = [None] * NCH
            for c in range(NCH):
                cs = slice(c * CH, (c + 1) * CH)
                h0[c] = wpool.tile([128, 2 * CH], BF16, tag="h0", bufs=4,
                                   name=f"h0i{c}")
                h1[c] = wpool.tile([128, 2 * CH], BF16, tag="h1", bufs=4,
                                   name=f"h1i{c}")
                for m in range(4):
                    ph = ppool.tile([128, CH], F32, tag="pn", bufs=4,
                                    name=f"pinit{c}_{m}")
                    mm(ph[:], wlat[:, m * 128:(m + 1) * 128], zT[:, cs],
                       True, True)
                    dst = (h0[c] if m < 2 else h1[c])
                    nc.scalar.activation(dst[:, (m % 2) * CH:(m % 2 + 1) * CH],
                                         ph[:], AF.Identity,
                                         bias=bias_ap(16 + m))

            def emit_head(t, h1s):
                for c in range(NCH):
                    cs = slice(c * CH, (c + 1) * CH)
                    hdd = wpool.tile([128, 2 * CH], BF16, tag="hdd", bufs=4,
                                     name=f"hdd{t}_{c}")
                    phds = []
                    for m in range(2):
                        ms = slice(m * 128, (m + 1) * 128)
                        phd = ppool.tile([128, CH], F32, tag="pr", bufs=2,
                                         name=f"phd{t}{c}{m}")
                        mm(phd[:], w1[0][:, ms], h1s[c][:, 0:CH], True, False)
                        mm(phd[:], w1[1][:, ms], h1s[c][:, CH:2 * CH], False, True)
                        phds.append(phd)
                    for m in range(2):
                        nc.scalar.activation(hdd[:, m * CH:(m + 1) * CH],
                                             phds[m][:], AF.Relu,
                                             bias=bias_ap(12 + m))
                    pl0 = ppool.tile([128, CH], F32, tag="pn", bufs=4,
                                     name=f"pl0{t}{c}")
                    mm(pl0[:], w2[0][:, 0:128], hdd[:, 0:CH], True, False)
                    mm(pl0[:], w2[1][:, 0:128], hdd[:, CH:2 * CH], False, True)
                    pl1 = ppool.tile([VOCAB - 128, CH], F32, tag="pn", bufs=4,
                                     name=f"pl1{t}{c}")
                    mm(pl1[:], w2[0][:, 128:VOCAB], hdd[:, 0:CH], True, False)
                    mm(pl1[:], w2[1][:, 128:VOCAB], hdd[:, CH:2 * CH], False, True)
                    lg0 = iopool.tile([128, CH], F32, tag="lg0",
                                      name=f"lg0{t}{c}")
                    lg1 = iopool.tile([VOCAB - 128, CH], F32, tag="lg1",
                                      name=f"lg1{t}{c}")
                    nc.scalar.activation(lg0[:], pl0[:], AF.Identity,
                                         bias=bias_ap(14))
                    nc.scalar.activation(lg1[:], pl1[:], AF.Identity,
                                         bias=bias_ap(15, rows=VOCAB - 128))
                    nc.sync.dma_start(out_d[t, 0:128, cs], lg0[:])
                    nc.sync.dma_start(out_d[t, 128:VOCAB, cs], lg1[:])

            pending_head = None

            # ---- time loop ----
            # Emission order = per-engine execution order. Emit chunk c's
            # matmuls, then its gate chain; chunk c+1's matmuls fill the PE
            # while chunk c's ACT/DVE chain runs. z-gate PSUM groups are
            # emitted last within a chunk (z is needed late) to cut peak
            # PSUM pressure.
            for t in range(n_steps):
                ohs = []
                for c in range(NCH):
                    cs = slice(c * CH, (c + 1) * CH)
                    if fp8_embed:
                        ohc = iopool.tile([74, 2 * CH], FP8, tag="oha",
                                          name=f"oh{t}_{c}")
                        nc.sync.dma_start(
                            ohc[:], oh_d[t].rearrange(
                                "k (j b) -> k j b", j=2)[:, :, cs])
                        if hybrid_n:
                            ohA = iopool.tile([128, CH], BF16, tag="ohA",
                                              name=f"ohA{t}_{c}")
                            nc.sync.dma_start(ohA[:], ohbf_d[t, 0:128, cs])
                            ohB = iopool.tile([VOCAB - 128, CH], BF16, tag="ohB",
                                              name=f"ohB{t}_{c}")
                            nc.sync.dma_start(ohB[:], ohbf_d[t, 128:VOCAB, cs])
                            ohs.append((ohc, (ohA, ohB)))
                        else:
                            ohs.append((ohc, None))
                    else:
                        oha = iopool.tile([128, CH], BF16, tag="oha",
                                          name=f"oha{t}_{c}")
                        nc.sync.dma_start(oha[:], oh_d[t, 0:128, cs])
                        ohb = iopool.tile([VOCAB - 128, CH], BF16, tag="ohb",
                                          name=f"ohb{t}_{c}")
                        nc.sync.dma_start(ohb[:], oh_d[t, 128:VOCAB, cs])
                        ohs.append((oha, ohb))

                h0new = [None] * NCH
                for layer in range(2):
                    if layer == 1 and pending_head is not None:
                        emit_head(*pending_head)
                        pending_head = None
                    if layer == 0:
                        wh = whh0
                        sigc, tanc, bhnc = 0, (4, 5), (20, 21)
                    else:
                        wh = whh1
                        sigc, tanc, bhnc = 6, (10, 11), (22, 23)

                    for c in range(NCH):
                        hprev = h0[c] if layer == 0 else h1[c]
                        use_dr = fp8_embed and layer == 0
                        if layer == 0:
                            if not fp8_embed:
                                ia, ib = emb2a, emb2b
                                ra, rb = ohs[c]
                            else:
                                oh_rhs = ohs[c][0].rearrange("k (j b) -> k j b", j=2)
                                if hybrid_n:
                                    ia, ib = emb2a, emb2b
                                    ra, rb = ohs[c][1]
                        else:
                            ia, ib = wih1[0], wih1[1]
                            ra = h0new[c][:, 0:CH]
                            rb = h0new[c][:, CH:2 * CH]

                        def mm_gi(pg, gs, start, stop, dr=True):
                            # gi contribution for gate rows gs
                            if use_dr and dr:
                                lhs = emb2dr.rearrange(
                                    "k (j m) -> k j m", j=2)[:, :, gs]
                                nc.tensor.matmul(pg, lhs, oh_rhs,
                                                 start=start, stop=stop,
                                                 perf_mode=DR)
                            else:
                                mm(pg, ia[:, gs], ra, start, False)
                                mm(pg, ib[:, gs], rb, False, stop)

                        def grp4(pg, gs):
                            mm(pg[:], wh[0][:, gs], hprev[:, 0:CH], True, False)
                            mm(pg[:], wh[1][:, gs], hprev[:, CH:2 * CH], False, False)
                            mm_gi(pg[:], gs, False, True)

                        # r first, then n-gate psum, z last
                        pr, pin, phn, pz = [], [], [], []
                        for g in range(2):
                            pg = ppool.tile([128, CH], F32, tag="pr", bufs=2,
                                            name=f"pr{t}{c}{layer}{g}")
                            grp4(pg, slice(g * 128, (g + 1) * 128))
                            pr.append(pg)
                        for g in range(2):
                            pg = ppool.tile([128, CH], F32, tag="pz", bufs=2,
                                            name=f"pz{t}{c}{layer}{g}")
                            grp4(pg, slice((2 + g) * 128, (3 + g) * 128))
                            pz.append(pg)

                        for g in range(2):
                            gs = slice((4 + g) * 128, (5 + g) * 128)
                            pi = ppool.tile([128, CH], F32, tag="pn", bufs=4,
                                            name=f"pi{t}{c}{layer}{g}")
                            mm_gi(pi[:], gs, True, True,
                                  dr=not (fp8_embed and hybrid_n))
                            pin.append(pi)
                            pp = ppool.tile([128, CH], F32, tag="pn", bufs=4,
                                            name=f"pp{t}{c}{layer}{g}")
                            mm(pp[:], wh[0][:, gs], hprev[:, 0:CH], True, False)
                            mm(pp[:], wh[1][:, gs], hprev[:, CH:2 * CH], False, True)
                            phn.append(pp)
                        # ---- gate chain (ACT + DVE), in dependency order ----
                        rg, zg = [], []
                        for g in range(2):
                            r_ = wpool.tile([128, CH], BF16, tag="r", bufs=3,
                                            name=f"r{t}{c}{layer}{g}")
                            nc.scalar.activation(r_[:], pr[g][:], AF.Sigmoid,
                                                 bias=bias_ap(sigc + g))
                            rg.append(r_)
                        tmps, npres = [], []
                        for g in range(2):
                            tmp = wpool.tile([128, CH], BF16, tag="tmp", bufs=3,
                                             name=f"tm{t}{c}{layer}{g}")
                            if zero_bias:
                                nc.vector.tensor_mul(tmp[:], rg[g][:], phn[g][:])
                            else:
                                nc.vector.scalar_tensor_tensor(
                                    tmp[:], phn[g][:], bias_ap(bhnc[g]),
                                    rg[g][:], OP.add, OP.mult)
                            npre = wpool.tile([128, CH], BF16, tag="npre", bufs=3,
                                              name=f"np{t}{c}{layer}{g}")
                            nc.vector.tensor_add(npre[:], tmp[:], pin[g][:])
                            npres.append(npre)
                        for g in range(2):
                            z_ = wpool.tile([128, CH], BF16, tag="z", bufs=3,
                                            name=f"z{t}{c}{layer}{g}")
                            nc.scalar.activation(z_[:], pz[g][:], AF.Sigmoid,
                                                 bias=bias_ap(sigc + 2 + g))
                            zg.append(z_)
                        ns_ = []
                        for g in range(2):
                            n_ = wpool.tile([128, CH], BF16, tag="n", bufs=3,
                                            name=f"n{t}{c}{layer}{g}")
                            nc.scalar.activation(n_[:], npres[g][:], AF.Tanh,
                                                 bias=bias_ap(tanc[g]))
                            ns_.append(n_)
                        hn = wpool.tile([128, 2 * CH], BF16,
                                        tag=("h0" if layer == 0 else "h1"),
                                        bufs=4, name=f"h{layer}_{t}_{c}")
                        for g in range(2):
                            d_ = wpool.tile([128, CH], BF16, tag="d", bufs=3,
                                            name=f"d{t}{c}{layer}{g}")
                            nc.vector.tensor_sub(d_[:], hprev[:, g * CH:(g + 1) * CH],
                                                 ns_[g][:])
                            e_ = wpool.tile([128, CH], BF16, tag="e", bufs=3,
                                            name=f"e{t}{c}{layer}{g}")
                            nc.vector.tensor_mul(e_[:], zg[g][:], d_[:])
                            nc.vector.tensor_add(hn[:, g * CH:(g + 1) * CH],
                                                 ns_[g][:], e_[:])
                        if layer == 0:
                            h0new[c] = hn
                            h0[c] = hn
                        else:
                            h1[c] = hn

                pending_head = (t, [h1[0], h1[1]])
            if pending_head is not None:
                emit_head(*pending_head)
                pending_head = None

    nc.compile()
    return nc


def _host_prep(z, target_tokens, emb, W_lat, b_lat,
               W_ih0, W_hh0, b_ih0, b_hh0,
               W_ih1, W_hh1, b_ih1, b_hh1,
               W1, b1, W2, b2, n_steps=NSTEPS, fp8_embed=True, hybrid_n=True):
    """Build per-core input maps (all float32)."""
    f = np.float32
    z = np.asarray(z, f)
    tt = np.asarray(target_tokens)
    emb = np.asarray(emb, f)
    W_lat = np.asarray(W_lat, f)

    # teacher-forced input tokens: [START, tgt[:,1], ..., tgt[:,T-2]]
    tokens_in = np.concatenate(
        [np.full((B, 1), START_IDX, dtype=np.int64),
         np.asarray(tt[:, 1:T - 1], np.int64)], axis=1)  # [B, 49]
    tokens_in = tokens_in[:, :n_steps]

    emb2 = (emb @ np.asarray(W_ih0, f).T).astype(f)        # [VOCAB, 768]

    # bias packing: 24 columns
    bias = np.zeros((128, 24), f)
    b_ih0 = np.asarray(b_ih0, f); b_hh0 = np.asarray(b_hh0, f)
    b_ih1 = np.asarray(b_ih1, f); b_hh1 = np.asarray(b_hh1, f)
    sig0 = (b_ih0 + b_hh0)[:512].reshape(4, 128)
    sig1 = (b_ih1 + b_hh1)[:512].reshape(4, 128)
    for j in range(4):
        bias[:, j] = sig0[j]
        bias[:, 6 + j] = sig1[j]
    bias[:, 4] = b_ih0[512:640]; bias[:, 5] = b_ih0[640:768]
    bias[:, 10] = b_ih1[512:640]; bias[:, 11] = b_ih1[640:768]
    b1 = np.asarray(b1, f); b2 = np.asarray(b2, f)
    bias[:, 12] = b1[:128]; bias[:, 13] = b1[128:]
    bias[:, 14] = b2[:128]; bias[:VOCAB - 128, 15] = b2[128:]
    b_lat = np.asarray(b_lat, f)
    for j in range(4):
        bias[:, 16 + j] = b_lat[j * 128:(j + 1) * 128]
    bias[:, 20] = b_hh0[512:640]; bias[:, 21] = b_hh0[640:768]
    bias[:, 22] = b_hh1[512:640]; bias[:, 23] = b_hh1[640:768]

    import ml_dtypes
    bf16 = ml_dtypes.bfloat16
    wpack = np.zeros((128, PACK_COLS), bf16)

    def put(name, arr, rows=128):
        o, w = PACK_OFF[name]
        wpack[:rows, o:o + w] = arr.astype(bf16)

    whh0T = np.asarray(W_hh0, f).T
    wih1T = np.asarray(W_ih1, f).T
    whh1T = np.asarray(W_hh1, f).T
    w1T = np.asarray(W1, f).T
    w2T = np.asarray(W2, f).T
    put("emb2a", emb2[0:128])
    put("emb2b", emb2[128:VOCAB], rows=VOCAB - 128)
    put("whh0k0", whh0T[0:128]); put("whh0k1", whh0T[128:256])
    put("wih1k0", wih1T[0:128]); put("wih1k1", wih1T[128:256])
    put("whh1k0", whh1T[0:128]); put("whh1k1", whh1T[128:256])
    put("w1k0", w1T[0:128]); put("w1k1", w1T[128:256])
    put("w2k0", w2T[0:128]); put("w2k1", w2T[128:256])
    put("wlat", W_lat.T, rows=LATENT)
    put("biases", bias)

    if fp8_embed:
        import ml_dtypes as _md
        fp8 = _md.float8_e4m3
        # scale table up, one-hot down by an exact power of two: keeps the
        # product identical while lifting table entries out of fp8 subnormals
        emb2dr = np.zeros((74, 2, G3), np.float32)
        emb2dr[:, 0, :] = emb2[0::2][:74]
        emb2dr[:, 1, :] = emb2[1::2][:74]
        emb2dr = (emb2dr * 64.0).reshape(74, 2 * G3).astype(fp8)

    in_maps = []
    zo, zw = PACK_OFF["zT"]
    for core in range(NCORES):
        rows = slice(core * BL, (core + 1) * BL)
        tok = tokens_in[rows]                      # [BL, n_steps]
        tsteps = np.arange(n_steps)[None, :].repeat(BL, 0)   # [BL, n_steps]
        bidx = np.arange(BL)[:, None].repeat(n_steps, 1)
        wp = wpack.copy()
        wp[:LATENT, zo:zo + zw] = z[rows].T.astype(bf16)
        m = {"wpack": wp}
        if fp8_embed:
            # oh[t, ki, j*BL + b] = (tok[b,t] == 2*ki + j)
            oh = np.zeros((n_steps, 74, 2, BL), np.float32)
            oh[tsteps.ravel(), (tok // 2).ravel(), (tok % 2).ravel(),
               bidx.ravel()] = 1.0 / 64.0
            m["oh"] = oh.reshape(n_steps, 74, 2 * BL).astype(fp8)
            m["emb2dr"] = emb2dr
            if hybrid_n:
                ohb_ = np.zeros((n_steps, VOCAB, BL), f)
                ohb_[tsteps.ravel(), tok.ravel(), bidx.ravel()] = 1.0
                m["ohbf"] = ohb_.astype(bf16)
        else:
            oh = np.zeros((n_steps, VOCAB, BL), f)
            oh[tsteps.ravel(), tok.ravel(), bidx.ravel()] = 1.0
            m["oh"] = oh.astype(bf16)
        in_maps.append(m)
    return in_maps


class _Runner:
    """Compile once; run many times with device-resident inputs (no
    donation) so repeated calls time the NEFF execution itself."""

    def __init__(self, n_steps=NSTEPS, zero_bias=True, fp8_embed=True, hybrid_n=True):
        import jax
        import numpy as _np
        from jax.sharding import Mesh, PartitionSpec, NamedSharding
        from jax.experimental.shard_map import shard_map
        import concourse.bass2jax as b2j
        import concourse.mybir as mybir

        nc = _build_graph(n_steps, zero_bias=zero_bias, fp8_embed=fp8_embed,
                          hybrid_n=hybrid_n)
        self.fp8_embed = fp8_embed
        self.hybrid_n = hybrid_n
        b2j.install_neuronx_cc_hook()
        self.nc = nc
        self.n_steps = n_steps

        partition_name = (nc.partition_id_tensor.name
                          if nc.partition_id_tensor else None)
        in_names, out_names, out_avals, zero_outs = [], [], [], []
        for alloc in nc.m.functions[0].allocations:
            if not isinstance(alloc, mybir.MemoryLocationSet):
                continue
            name = alloc.memorylocations[0].name
            if alloc.kind == "ExternalInput":
                if name != partition_name:
                    in_names.append(name)
            elif alloc.kind == "ExternalOutput":
                shape = list(alloc.tensor_shape)
                out_avals.append(jax.core.ShapedArray(shape, _np.float32))
                out_names.append(name)
                zero_outs.append(_np.zeros(shape, _np.float32))
        self.in_names, self.out_names = list(in_names), out_names
        bind_names = list(in_names) + list(out_names)
        if partition_name is not None:
            bind_names.append(partition_name)

        def _body(*args):
            operands = list(args)
            if partition_name is not None:
                operands.append(b2j.partition_id_tensor())
            outs = b2j._bass_exec_p.bind(
                *operands,
                out_avals=tuple(out_avals),
                in_names=tuple(bind_names),
                out_names=tuple(out_names),
                lowering_input_output_aliases=(),
                sim_require_finite=True,
                sim_require_nnan=True,
                nc=nc,
            )
            return tuple(outs)

        devices = jax.devices()[:NCORES]
        mesh = Mesh(np.asarray(devices), ("core",))
        nin = len(in_names) + len(zero_outs)
        self._fn = jax.jit(shard_map(
            _body, mesh=mesh,
            in_specs=(PartitionSpec("core"),) * nin,
            out_specs=(PartitionSpec("core"),) * len(out_names),
            check_rep=False), keep_unused=True)
        self._sharding = NamedSharding(mesh, PartitionSpec("core"))
        self._jax = jax
        self._zero_outs = zero_outs
        self._placed = None

    def place(self, in_maps):
        """Transfer concatenated per-core inputs to the devices once."""
        jax = self._jax
        concat = []
        for name in self.in_names:
            arr = np.concatenate([m[name] for m in in_maps], axis=0)
            concat.append(jax.device_put(arr, self._sharding))
        for z in self._zero_outs:
            zz = np.zeros((NCORES * z.shape[0], *z.shape[1:]), z.dtype)
            concat.append(jax.device_put(zz, self._sharding))
        self._placed = concat

    def run(self):
        outs = self._fn(*self._placed)
        return outs

    def run_blocked(self):
        outs = self._fn(*self._placed)
        for o in outs:
            o.block_until_ready()
        return outs


def _assemble_logits(out_concat, n_steps):
    """out_concat: [NCORES*n_steps, VOCAB, BL] -> [B, n_steps, VOCAB]."""
    o = np.asarray(out_concat).reshape(NCORES, n_steps, VOCAB, BL)
    # [core, t, v, b] -> [core, b, t, v]
    return o.transpose(0, 3, 1, 2).reshape(B, n_steps, VOCAB)


def kernel(z, target_tokens, emb, W_lat, b_lat,
           W_ih0, W_hh0, b_ih0, b_hh0,
           W_ih1, W_hh1, b_ih1, b_hh1,
           W1, b1, W2, b2, _n_steps=NSTEPS, _runner=None):
    if _runner is None:
        zb = all(np.allclose(np.asarray(b), 0.0) for b in
                 (b_lat, b_ih0, b_hh0, b_ih1, b_hh1, b1, b2))
        _runner = _Runner(_n_steps, zero_bias=zb)
    r = _runner
    in_maps = _host_prep(z, target_tokens, emb, W_lat, b_lat,
                         W_ih0, W_hh0, b_ih0, b_hh0,
                         W_ih1, W_hh1, b_ih1, b_hh1,
                         W1, b1, W2, b2, n_steps=_n_steps,
                         fp8_embed=getattr(r, "fp8_embed", True),
                         hybrid_n=getattr(r, "hybrid_n", True))
    r.place(in_maps)
    outs = r.run_blocked()
    logits = _assemble_logits(outs[r.out_names.index("out")], _n_steps)
    generated = np.asarray(target_tokens)[:, 1:]
    return logits, generated


# revision 40
# speedup vs baseline: 101.7631x; 1.0211x over previous
"""Trainium2 Bass kernel for nn_AutoregressiveFormulaDecoder.

2-layer GRU decoder (HID=256) with teacher forcing + fused MLP head.
Pure data parallel over 8 NeuronCores: 1024 batch rows per core, no
collectives; host shards inputs and reassembles the output.

Device layout is "transposed" (features on SBUF partitions, batch on the
free dim) so weights are the PE-stationary operand and per-feature biases
are per-partition ACT biases.

Per step (49 steps), per 512-wide batch chunk:
  - the input-side projection gi0 = (emb @ W_ih0.T)[token] is computed as
    a one-hot matmul; for the r/z gates it runs in fp8e4m3 with
    perf_mode=DoubleRow (the 148-deep vocab contraction folds to one
    74-partition MM at 0.5 cyc/row; one-hot entries are exact in fp8 and
    the table/one-hot carry a 64 / 1-64 exact power-of-two rescale).
    The tanh-path i_n keeps bf16 one-hot matmuls - fp8 there doubles the
    end-to-end error while r/z fp8 contributes no measurable error.
  - r/z gates: gi and gh matmuls ACCUMULATE in one PSUM bank, then a
    single Sigmoid with fused per-partition bias reads PSUM directly.
  - n gate: i_n / h_n in separate PSUM banks; DVE r*h_n + i_n, Tanh.
  - h' = n + z*(h - n) as all-bf16 SBUF tensor ops (DVE 2x mode).
  - MLP head (relu(W1@h1), W2@...) is fused, software-pipelined one step
    behind the recurrence.

Scheduling notes (these drive the performance):
  - Tile engines execute their instruction streams IN EMISSION ORDER, so
    chunk c+1's matmuls are emitted between chunk c's matmuls and the
    ops that consume them - the PE stays busy while ACT/DVE run chunk
    c's gate chain.
  - PSUM tiles are tagged by drain class (pr=2, pn=4, pz=2 banks) so a
    new group's bank-reuse waits on an ACT/DVE op that actually fires
    early, not an arbitrary late one.
  - All constants ship in ONE packed bf16 DRAM tensor (single DMA):
    walrus allows at most 1 sync-wait per instruction, so fan-in from
    many DMA queues must be avoided.

Cost-model (CoreSim) predicted exec: ~1.065 ms for the full 49 steps
(PE-busy floor for this decomposition is ~1.047 ms -> 98% PE occupancy),
rel err ~6.4e-3 (sim) / 6.7e-3 (hardware) vs the float32 reference.
"""

import numpy as np

VOCAB = 148
START_IDX = 1
LATENT = 32
HID = 256
G3 = 3 * HID  # 768
B = 8192
T = 50
NSTEPS = T - 1  # 49
NCORES = 8
BL = B // NCORES  # 1024 batch rows per core
CH = 512          # batch chunk (one PSUM bank of f32)
NCH = BL // CH    # 2


# packed constant layout: name -> (col offset, col width); all float32 columns
_PACK_SPEC = [
    ("emb2a", G3), ("emb2b", G3),
    ("whh0k0", G3), ("whh0k1", G3),
    ("wih1k0", G3), ("wih1k1", G3),
    ("whh1k0", G3), ("whh1k1", G3),
    ("w1k0", HID), ("w1k1", HID),
    ("w2k0", VOCAB), ("w2k1", VOCAB),
    ("wlat", 2 * HID), ("zT", BL), ("biases", 24),
]
PACK_OFF = {}
_o = 0
for _n, _w in _PACK_SPEC:
    PACK_OFF[_n] = (_o, _w)
    _o += _w
PACK_COLS = _o


def _build_graph(n_steps=NSTEPS, zero_bias=True, fp8_embed=True, hybrid_n=True):
    import concourse.bass as bass
    import concourse.bacc as bacc
    import concourse.mybir as mybir
    import concourse.tile as tile

    F32 = mybir.dt.float32
    BF16 = mybir.dt.bfloat16
    FP8 = mybir.dt.float8e4
    DR = mybir.MatmulPerfMode.DoubleRow
    AF = mybir.ActivationFunctionType
    OP = mybir.AluOpType

    nc = bacc.Bacc()

    if fp8_embed:
        oh_d = nc.declare_dram_parameter("oh", [n_steps, 74, 2 * BL], FP8,
                                         isOutput=False)
        emb2dr_d = nc.declare_dram_parameter("emb2dr", [74, 2 * G3], FP8,
                                             isOutput=False)
        if hybrid_n:
            ohbf_d = nc.declare_dram_parameter("ohbf", [n_steps, VOCAB, BL],
                                               BF16, isOutput=False)
    else:
        oh_d = nc.declare_dram_parameter("oh", [n_steps, VOCAB, BL], BF16,
                                         isOutput=False)
    wpack_d = nc.declare_dram_parameter("wpack", [128, PACK_COLS], BF16, isOutput=False)
    out_d = nc.declare_dram_parameter("out", [n_steps, VOCAB, BL], F32, isOutput=True)

    with tile.TileContext(nc) as tc:
        with (
            tc.tile_pool(name="const", bufs=1) as cpool,
            tc.tile_pool(name="io", bufs=6) as iopool,
            tc.tile_pool(name="work", bufs=2) as wpool,
            tc.tile_pool(name="psum", bufs=1, space="PSUM") as ppool,
        ):
            # ---- one DMA for every constant ----
            wpk = cpool.tile([128, PACK_COLS], BF16)
            nc.sync.dma_start(wpk[:], wpack_d[:, :])
            if fp8_embed:
                emb2dr = cpool.tile([74, 2 * G3], FP8)
                nc.sync.dma_start(emb2dr[:], emb2dr_d[:, :])

            def P(name, rows=128):
                o, w = PACK_OFF[name]
                return wpk[0:rows, o:o + w]

            emb2a = P("emb2a")
            emb2b = P("emb2b", rows=VOCAB - 128)
            whh0 = [P("whh0k0"), P("whh0k1")]
            wih1 = [P("wih1k0"), P("wih1k1")]
            whh1 = [P("whh1k0"), P("whh1k1")]
            w1 = [P("w1k0"), P("w1k1")]
            w2 = [P("w2k0"), P("w2k1")]
            wlat = P("wlat", rows=LATENT)
            zT = P("zT", rows=LATENT)

            def bias_ap(col, rows=128):
                o, _ = PACK_OFF["biases"]
                return wpk[0:rows, o + col:o + col + 1]

            def mm(pt, lhsT, rhs, start, stop):
                nc.tensor.matmul(pt, lhsT, rhs, start=start, stop=stop)

            # ---- init hidden state: hT = W_lat @ zT + b_lat ----
            h0 = [None] * NCH   # wide [128, (k,512)] bf16 per chunk
            h1 = [None] * NCH
            for c in range(NCH):
                cs = slice(c * CH, (c + 1) * CH)
                h0[c] = wpool.tile([128, 2 * CH], BF16, tag="h0", bufs=6,
                                   name=f"h0i{c}")
                h1[c] = wpool.tile([128, 2 * CH], BF16, tag="h1", bufs=6,
                                   name=f"h1i{c}")
                for m in range(4):
                    ph = ppool.tile([128, CH], F32, tag="pn", bufs=4,
                                    name=f"pinit{c}_{m}")
                    mm(ph[:], wlat[:, m * 128:(m + 1) * 128], zT[:, cs],
                       True, True)
                    dst = (h0[c] if m < 2 else h1[c])
                    nc.scalar.activation(dst[:, (m % 2) * CH:(m % 2 + 1) * CH],
                                         ph[:], AF.Identity,
                                         bias=bias_ap(16 + m))

            def emit_head(t, h1s):
                for c in range(NCH):
                    cs = slice(c * CH, (c + 1) * CH)
                    hdd = wpool.tile([128, 2 * CH], BF16, tag="hdd", bufs=6,
                                     name=f"hdd{t}_{c}")
                    phds = []
                    for m in range(2):
                        ms = slice(m * 128, (m + 1) * 128)
                        phd = ppool.tile([128, CH], F32, tag="pr", bufs=2,
                                         name=f"phd{t}{c}{m}")
                        mm(phd[:], w1[0][:, ms], h1s[c][:, 0:CH], True, False)
                        mm(phd[:], w1[1][:, ms], h1s[c][:, CH:2 * CH], False, True)
                        phds.append(phd)
                    for m in range(2):
                        nc.scalar.activation(hdd[:, m * CH:(m + 1) * CH],
                                             phds[m][:], AF.Relu,
                                             bias=bias_ap(12 + m))
                    pl0 = ppool.tile([128, CH], F32, tag="pn", bufs=4,
                                     name=f"pl0{t}{c}")
                    mm(pl0[:], w2[0][:, 0:128], hdd[:, 0:CH], True, False)
                    mm(pl0[:], w2[1][:, 0:128], hdd[:, CH:2 * CH], False, True)
                    pl1 = ppool.tile([VOCAB - 128, CH], F32, tag="pn", bufs=4,
                                     name=f"pl1{t}{c}")
                    mm(pl1[:], w2[0][:, 128:VOCAB], hdd[:, 0:CH], True, False)
                    mm(pl1[:], w2[1][:, 128:VOCAB], hdd[:, CH:2 * CH], False, True)
                    lg0 = iopool.tile([128, CH], F32, tag="lg0",
                                      name=f"lg0{t}{c}")
                    lg1 = iopool.tile([VOCAB - 128, CH], F32, tag="lg1",
                                      name=f"lg1{t}{c}")
                    nc.scalar.activation(lg0[:], pl0[:], AF.Identity,
                                         bias=bias_ap(14))
                    nc.scalar.activation(lg1[:], pl1[:], AF.Identity,
                                         bias=bias_ap(15, rows=VOCAB - 128))
                    nc.sync.dma_start(out_d[t, 0:128, cs], lg0[:])
                    nc.sync.dma_start(out_d[t, 128:VOCAB, cs], lg1[:])

            pending_head = None

            # ---- time loop ----
            # Emission order = per-engine execution order. Emit chunk c's
            # matmuls, then its gate chain; chunk c+1's matmuls fill the PE
            # while chunk c's ACT/DVE chain runs. z-gate PSUM groups are
            # emitted last within a chunk (z is needed late) to cut peak
            # PSUM pressure.
            for t in range(n_steps):
                ohs = []
                for c in range(NCH):
                    cs = slice(c * CH, (c + 1) * CH)
                    if fp8_embed:
                        ohc = iopool.tile([74, 2 * CH], FP8, tag="oha",
                                          name=f"oh{t}_{c}")
                        nc.sync.dma_start(
                            ohc[:], oh_d[t].rearrange(
                                "k (j b) -> k j b", j=2)[:, :, cs])
                        if hybrid_n:
                            ohA = iopool.tile([128, CH], BF16, tag="ohA",
                                              name=f"ohA{t}_{c}")
                            nc.sync.dma_start(ohA[:], ohbf_d[t, 0:128, cs])
                            ohB = iopool.tile([VOCAB - 128, CH], BF16, tag="ohB",
                                              name=f"ohB{t}_{c}")
                            nc.sync.dma_start(ohB[:], ohbf_d[t, 128:VOCAB, cs])
                            ohs.append((ohc, (ohA, ohB)))
                        else:
                            ohs.append((ohc, None))
                    else:
                        oha = iopool.tile([128, CH], BF16, tag="oha",
                                          name=f"oha{t}_{c}")
                        nc.sync.dma_start(oha[:], oh_d[t, 0:128, cs])
                        ohb = iopool.tile([VOCAB - 128, CH], BF16, tag="ohb",
                                          name=f"ohb{t}_{c}")
                        nc.sync.dma_start(ohb[:], oh_d[t, 128:VOCAB, cs])
                        ohs.append((oha, ohb))

                h0new = [None] * NCH
                for layer in range(2):
                    if layer == 1 and pending_head is not None:
                        emit_head(*pending_head)
                        pending_head = None
                    if layer == 0:
                        wh = whh0
                        sigc, tanc, bhnc = 0, (4, 5), (20, 21)
                    else:
                        wh = whh1
                        sigc, tanc, bhnc = 6, (10, 11), (22, 23)

                    for c in range(NCH):
                        hprev = h0[c] if layer == 0 else h1[c]
                        use_dr = fp8_embed and layer == 0
                        if layer == 0:
                            if not fp8_embed:
                                ia, ib = emb2a, emb2b
                                ra, rb = ohs[c]
                            else:
                                oh_rhs = ohs[c][0].rearrange("k (j b) -> k j b", j=2)
                                if hybrid_n:
                                    ia, ib = emb2a, emb2b
                                    ra, rb = ohs[c][1]
                        else:
                            ia, ib = wih1[0], wih1[1]
                            ra = h0new[c][:, 0:CH]
                            rb = h0new[c][:, CH:2 * CH]

                        def mm_gi(pg, gs, start, stop, dr=True):
                            # gi contribution for gate rows gs
                            if use_dr and dr:
                                lhs = emb2dr.rearrange(
                                    "k (j m) -> k j m", j=2)[:, :, gs]
                                nc.tensor.matmul(pg, lhs, oh_rhs,
                                                 start=start, stop=stop,
                                                 perf_mode=DR)
                            else:
                                mm(pg, ia[:, gs], ra, start, False)
                                mm(pg, ib[:, gs], rb, False, stop)

                        def grp4(pg, gs):
                            mm(pg[:], wh[0][:, gs], hprev[:, 0:CH], True, False)
                            mm(pg[:], wh[1][:, gs], hprev[:, CH:2 * CH], False, False)
                            mm_gi(pg[:], gs, False, True)

                        # emission order: r (chain head), z, then the n-gate
                        # pairs - this ordering measured fastest end-to-end
                        pr, pin, phn, pz = [], [], [], []
                        for g in range(2):
                            pg = ppool.tile([128, CH], F32, tag="pr", bufs=2,
                                            name=f"pr{t}{c}{layer}{g}")
                            grp4(pg, slice(g * 128, (g + 1) * 128))
                            pr.append(pg)
                        for g in range(2):
                            pg = ppool.tile([128, CH], F32, tag="pz", bufs=2,
                                            name=f"pz{t}{c}{layer}{g}")
                            grp4(pg, slice((2 + g) * 128, (3 + g) * 128))
                            pz.append(pg)
                        for g in range(2):
                            gs = slice((4 + g) * 128, (5 + g) * 128)
                            pi = ppool.tile([128, CH], F32, tag="pn", bufs=4,
                                            name=f"pi{t}{c}{layer}{g}")
                            mm_gi(pi[:], gs, True, True,
                                  dr=not (fp8_embed and hybrid_n))
                            pin.append(pi)
                            pp = ppool.tile([128, CH], F32, tag="pn", bufs=4,
                                            name=f"pp{t}{c}{layer}{g}")
                            mm(pp[:], wh[0][:, gs], hprev[:, 0:CH], True, False)
                            mm(pp[:], wh[1][:, gs], hprev[:, CH:2 * CH], False, True)
                            phn.append(pp)
                        # ---- gate chain (ACT + DVE), in dependency order ----
                        rg, zg = [], []
                        for g in range(2):
                            r_ = wpool.tile([128, CH], BF16, tag="r", bufs=4,
                                            name=f"r{t}{c}{layer}{g}")
                            nc.scalar.activation(r_[:], pr[g][:], AF.Sigmoid,
                                                 bias=bias_ap(sigc + g))
                            rg.append(r_)
                        tmps, npres = [], []
                        for g in range(2):
                            tmp = wpool.tile([128, CH], BF16, tag="tmp", bufs=4,
                                             name=f"tm{t}{c}{layer}{g}")
                            if zero_bias:
                                nc.vector.tensor_mul(tmp[:], rg[g][:], phn[g][:])
                            else:
                                nc.vector.scalar_tensor_tensor(
                                    tmp[:], phn[g][:], bias_ap(bhnc[g]),
                                    rg[g][:], OP.add, OP.mult)
                            npre = wpool.tile([128, CH], BF16, tag="npre", bufs=4,
                                              name=f"np{t}{c}{layer}{g}")
                            nc.vector.tensor_add(npre[:], tmp[:], pin[g][:])
                            npres.append(npre)
                        for g in range(2):
                            z_ = wpool.tile([128, CH], BF16, tag="z", bufs=4,
                                            name=f"z{t}{c}{layer}{g}")
                            nc.scalar.activation(z_[:], pz[g][:], AF.Sigmoid,
                                                 bias=bias_ap(sigc + 2 + g))
                            zg.append(z_)
                        ns_ = []
                        for g in range(2):
                            n_ = wpool.tile([128, CH], BF16, tag="n", bufs=4,
                                            name=f"n{t}{c}{layer}{g}")
                            nc.scalar.activation(n_[:], npres[g][:], AF.Tanh,
                                                 bias=bias_ap(tanc[g]))
                            ns_.append(n_)
                        hn = wpool.tile([128, 2 * CH], BF16,
                                        tag=("h0" if layer == 0 else "h1"),
                                        bufs=6, name=f"h{layer}_{t}_{c}")
                        for g in range(2):
                            d_ = wpool.tile([128, CH], BF16, tag="d", bufs=4,
                                            name=f"d{t}{c}{layer}{g}")
                            nc.vector.tensor_sub(d_[:], hprev[:, g * CH:(g + 1) * CH],
                                                 ns_[g][:])
                            e_ = wpool.tile([128, CH], BF16, tag="e", bufs=4,
                                            name=f"e{t}{c}{layer}{g}")
                            nc.vector.tensor_mul(e_[:], zg[g][:], d_[:])
                            nc.vector.tensor_add(hn[:, g * CH:(g + 1) * CH],
                                                 ns_[g][:], e_[:])
                        if layer == 0:
                            h0new[c] = hn
                            h0[c] = hn
                        else:
                            h1[c] = hn

                pending_head = (t, [h1[0], h1[1]])
            if pending_head is not None:
                emit_head(*pending_head)
                pending_head = None

    nc.compile()
    return nc


def _host_prep(z, target_tokens, emb, W_lat, b_lat,
               W_ih0, W_hh0, b_ih0, b_hh0,
               W_ih1, W_hh1, b_ih1, b_hh1,
               W1, b1, W2, b2, n_steps=NSTEPS, fp8_embed=True, hybrid_n=True):
    """Build per-core input maps (all float32)."""
    f = np.float32
    z = np.asarray(z, f)
    tt = np.asarray(target_tokens)
    emb = np.asarray(emb, f)
    W_lat = np.asarray(W_lat, f)

    # teacher-forced input tokens: [START, tgt[:,1], ..., tgt[:,T-2]]
    tokens_in = np.concatenate(
        [np.full((B, 1), START_IDX, dtype=np.int64),
         np.asarray(tt[:, 1:T - 1], np.int64)], axis=1)  # [B, 49]
    tokens_in = tokens_in[:, :n_steps]

    emb2 = (emb @ np.asarray(W_ih0, f).T).astype(f)        # [VOCAB, 768]

    # bias packing: 24 columns
    bias = np.zeros((128, 24), f)
    b_ih0 = np.asarray(b_ih0, f); b_hh0 = np.asarray(b_hh0, f)
    b_ih1 = np.asarray(b_ih1, f); b_hh1 = np.asarray(b_hh1, f)
    sig0 = (b_ih0 + b_hh0)[:512].reshape(4, 128)
    sig1 = (b_ih1 + b_hh1)[:512].reshape(4, 128)
    for j in range(4):
        bias[:, j] = sig0[j]
        bias[:, 6 + j] = sig1[j]
    bias[:, 4] = b_ih0[512:640]; bias[:, 5] = b_ih0[640:768]
    bias[:, 10] = b_ih1[512:640]; bias[:, 11] = b_ih1[640:768]
    b1 = np.asarray(b1, f); b2 = np.asarray(b2, f)
    bias[:, 12] = b1[:128]; bias[:, 13] = b1[128:]
    bias[:, 14] = b2[:128]; bias[:VOCAB - 128, 15] = b2[128:]
    b_lat = np.asarray(b_lat, f)
    for j in range(4):
        bias[:, 16 + j] = b_lat[j * 128:(j + 1) * 128]
    bias[:, 20] = b_hh0[512:640]; bias[:, 21] = b_hh0[640:768]
    bias[:, 22] = b_hh1[512:640]; bias[:, 23] = b_hh1[640:768]

    import ml_dtypes
    bf16 = ml_dtypes.bfloat16
    wpack = np.zeros((128, PACK_COLS), bf16)

    def put(name, arr, rows=128):
        o, w = PACK_OFF[name]
        wpack[:rows, o:o + w] = arr.astype(bf16)

    whh0T = np.asarray(W_hh0, f).T
    wih1T = np.asarray(W_ih1, f).T
    whh1T = np.asarray(W_hh1, f).T
    w1T = np.asarray(W1, f).T
    w2T = np.asarray(W2, f).T
    put("emb2a", emb2[0:128])
    put("emb2b", emb2[128:VOCAB], rows=VOCAB - 128)
    put("whh0k0", whh0T[0:128]); put("whh0k1", whh0T[128:256])
    put("wih1k0", wih1T[0:128]); put("wih1k1", wih1T[128:256])
    put("whh1k0", whh1T[0:128]); put("whh1k1", whh1T[128:256])
    put("w1k0", w1T[0:128]); put("w1k1", w1T[128:256])
    put("w2k0", w2T[0:128]); put("w2k1", w2T[128:256])
    put("wlat", W_lat.T, rows=LATENT)
    put("biases", bias)

    if fp8_embed:
        import ml_dtypes as _md
        fp8 = _md.float8_e4m3
        # scale table up, one-hot down by an exact power of two: keeps the
        # product identical while lifting table entries out of fp8 subnormals
        emb2dr = np.zeros((74, 2, G3), np.float32)
        emb2dr[:, 0, :] = emb2[0::2][:74]
        emb2dr[:, 1, :] = emb2[1::2][:74]
        emb2dr = (emb2dr * 64.0).reshape(74, 2 * G3).astype(fp8)

    in_maps = []
    zo, zw = PACK_OFF["zT"]
    for core in range(NCORES):
        rows = slice(core * BL, (core + 1) * BL)
        tok = tokens_in[rows]                      # [BL, n_steps]
        tsteps = np.arange(n_steps)[None, :].repeat(BL, 0)   # [BL, n_steps]
        bidx = np.arange(BL)[:, None].repeat(n_steps, 1)
        wp = wpack.copy()
        wp[:LATENT, zo:zo + zw] = z[rows].T.astype(bf16)
        m = {"wpack": wp}
        if fp8_embed:
            # oh[t, ki, j*BL + b] = (tok[b,t] == 2*ki + j)
            oh = np.zeros((n_steps, 74, 2, BL), np.float32)
            oh[tsteps.ravel(), (tok // 2).ravel(), (tok % 2).ravel(),
               bidx.ravel()] = 1.0 / 64.0
            m["oh"] = oh.reshape(n_steps, 74, 2 * BL).astype(fp8)
            m["emb2dr"] = emb2dr
            if hybrid_n:
                ohb_ = np.zeros((n_steps, VOCAB, BL), f)
                ohb_[tsteps.ravel(), tok.ravel(), bidx.ravel()] = 1.0
                m["ohbf"] = ohb_.astype(bf16)
        else:
            oh = np.zeros((n_steps, VOCAB, BL), f)
            oh[tsteps.ravel(), tok.ravel(), bidx.ravel()] = 1.0
            m["oh"] = oh.astype(bf16)
        in_maps.append(m)
    return in_maps


class _Runner:
    """Compile once; run many times with device-resident inputs (no
    donation) so repeated calls time the NEFF execution itself."""

    def __init__(self, n_steps=NSTEPS, zero_bias=True, fp8_embed=True, hybrid_n=True):
        import jax
        import numpy as _np
        from jax.sharding import Mesh, PartitionSpec, NamedSharding
        from jax.experimental.shard_map import shard_map
        import concourse.bass2jax as b2j
        import concourse.mybir as mybir

        nc = _build_graph(n_steps, zero_bias=zero_bias, fp8_embed=fp8_embed,
                          hybrid_n=hybrid_n)
        self.fp8_embed = fp8_embed
        self.hybrid_n = hybrid_n
        b2j.install_neuronx_cc_hook()
        self.nc = nc
        self.n_steps = n_steps

        partition_name = (nc.partition_id_tensor.name
                          if nc.partition_id_tensor else None)
        in_names, out_names, out_avals, zero_outs = [], [], [], []
        for alloc in nc.m.functions[0].allocations:
            if not isinstance(alloc, mybir.MemoryLocationSet):
                continue
            name = alloc.memorylocations[0].name
            if alloc.kind == "ExternalInput":
                if name != partition_name:
                    in_names.append(name)
            elif alloc.kind == "ExternalOutput":
                shape = list(alloc.tensor_shape)
                out_avals.append(jax.core.ShapedArray(shape, _np.float32))
                out_names.append(name)
                zero_outs.append(_np.zeros(shape, _np.float32))
        self.in_names, self.out_names = list(in_names), out_names
        bind_names = list(in_names) + list(out_names)
        if partition_name is not None:
            bind_names.append(partition_name)

        def _body(*args):
            operands = list(args)
            if partition_name is not None:
                operands.append(b2j.partition_id_tensor())
            outs = b2j._bass_exec_p.bind(
                *operands,
                out_avals=tuple(out_avals),
                in_names=tuple(bind_names),
                out_names=tuple(out_names),
                lowering_input_output_aliases=(),
                sim_require_finite=True,
                sim_require_nnan=True,
                nc=nc,
            )
            return tuple(outs)

        devices = jax.devices()[:NCORES]
        mesh = Mesh(np.asarray(devices), ("core",))
        nin = len(in_names) + len(zero_outs)
        self._fn = jax.jit(shard_map(
            _body, mesh=mesh,
            in_specs=(PartitionSpec("core"),) * nin,
            out_specs=(PartitionSpec("core"),) * len(out_names),
            check_rep=False), keep_unused=True)
        self._sharding = NamedSharding(mesh, PartitionSpec("core"))
        self._jax = jax
        self._zero_outs = zero_outs
        self._placed = None

    def place(self, in_maps):
        """Transfer concatenated per-core inputs to the devices once."""
        jax = self._jax
        concat = []
        for name in self.in_names:
            arr = np.concatenate([m[name] for m in in_maps], axis=0)
            concat.append(jax.device_put(arr, self._sharding))
        for z in self._zero_outs:
            zz = np.zeros((NCORES * z.shape[0], *z.shape[1:]), z.dtype)
            concat.append(jax.device_put(zz, self._sharding))
        self._placed = concat

    def run(self):
        outs = self._fn(*self._placed)
        return outs

    def run_blocked(self):
        outs = self._fn(*self._placed)
        for o in outs:
            o.block_until_ready()
        return outs


def _assemble_logits(out_concat, n_steps):
    """out_concat: [NCORES*n_steps, VOCAB, BL] -> [B, n_steps, VOCAB]."""
    o = np.asarray(out_concat).reshape(NCORES, n_steps, VOCAB, BL)
    # [core, t, v, b] -> [core, b, t, v]
    return o.transpose(0, 3, 1, 2).reshape(B, n_steps, VOCAB)


def kernel(z, target_tokens, emb, W_lat, b_lat,
           W_ih0, W_hh0, b_ih0, b_hh0,
           W_ih1, W_hh1, b_ih1, b_hh1,
           W1, b1, W2, b2, _n_steps=NSTEPS, _runner=None):
    if _runner is None:
        zb = all(np.allclose(np.asarray(b), 0.0) for b in
                 (b_lat, b_ih0, b_hh0, b_ih1, b_hh1, b1, b2))
        _runner = _Runner(_n_steps, zero_bias=zb)
    r = _runner
    in_maps = _host_prep(z, target_tokens, emb, W_lat, b_lat,
                         W_ih0, W_hh0, b_ih0, b_hh0,
                         W_ih1, W_hh1, b_ih1, b_hh1,
                         W1, b1, W2, b2, n_steps=_n_steps,
                         fp8_embed=getattr(r, "fp8_embed", True),
                         hybrid_n=getattr(r, "hybrid_n", True))
    r.place(in_maps)
    outs = r.run_blocked()
    logits = _assemble_logits(outs[r.out_names.index("out")], _n_steps)
    generated = np.asarray(target_tokens)[:, 1:]
    return logits, generated


# revision 42
# speedup vs baseline: 105.0131x; 1.0319x over previous
"""Trainium2 Bass kernel for nn_AutoregressiveFormulaDecoder.

2-layer GRU decoder (HID=256) with teacher forcing + fused MLP head.
Pure data parallel over 8 NeuronCores: 1024 batch rows per core, no
collectives; host shards inputs and reassembles the output.

Device layout is "transposed" (features on SBUF partitions, batch on the
free dim) so weights are the PE-stationary operand and per-feature biases
are per-partition ACT biases.

Per step (49 steps), per 512-wide batch chunk:
  - the input-side projection gi0 = (emb @ W_ih0.T)[token] is computed as
    a one-hot matmul; for the r/z gates it runs in fp8e4m3 with
    perf_mode=DoubleRow (the 148-deep vocab contraction folds to one
    74-partition MM at 0.5 cyc/row; one-hot entries are exact in fp8 and
    the table/one-hot carry a 64 / 1-64 exact power-of-two rescale).
    The tanh-path i_n adds a SECOND accumulating DoubleRow matmul against
    an fp8 residual table fp8(emb2*64 - fp8(emb2*64)), cancelling ~94% of
    the fp8 quantisation error - accuracy matches the bf16 path.
  - r/z gates: gi and gh matmuls ACCUMULATE in one PSUM bank, then a
    single Sigmoid with fused per-partition bias reads PSUM directly.
  - n gate: i_n / h_n in separate PSUM banks; DVE r*h_n + i_n, Tanh.
  - h' = n + z*(h - n) as all-bf16 SBUF tensor ops (DVE 2x mode).
  - MLP head (relu(W1@h1), W2@...) is fused, software-pipelined one step
    behind the recurrence.

Scheduling notes (these drive the performance):
  - Tile engines execute their instruction streams IN EMISSION ORDER, so
    chunk c+1's matmuls are emitted between chunk c's matmuls and the
    ops that consume them - the PE stays busy while ACT/DVE run chunk
    c's gate chain.
  - PSUM tiles are tagged by drain class (pr=2, pn=4, pz=2 banks) so a
    new group's bank-reuse waits on an ACT/DVE op that actually fires
    early, not an arbitrary late one.
  - All constants ship in ONE packed bf16 DRAM tensor (single DMA):
    walrus allows at most 1 sync-wait per instruction, so fan-in from
    many DMA queues must be avoided.

Cost-model (CoreSim) predicted exec: ~1.032 ms for the full 49 steps
(PE-busy floor for this decomposition is ~1.006 ms -> 97.5% occupancy),
rel err ~6.3e-3 vs the float32 reference.
"""

import numpy as np

VOCAB = 148
START_IDX = 1
LATENT = 32
HID = 256
G3 = 3 * HID  # 768
B = 8192
T = 50
NSTEPS = T - 1  # 49
NCORES = 8
BL = B // NCORES  # 1024 batch rows per core
CH = 512          # batch chunk (one PSUM bank of f32)
NCH = BL // CH    # 2


# packed constant layout: name -> (col offset, col width); all float32 columns
_PACK_SPEC = [
    ("emb2a", G3), ("emb2b", G3),
    ("whh0k0", G3), ("whh0k1", G3),
    ("wih1k0", G3), ("wih1k1", G3),
    ("whh1k0", G3), ("whh1k1", G3),
    ("w1k0", HID), ("w1k1", HID),
    ("w2k0", VOCAB), ("w2k1", VOCAB),
    ("wlat", 2 * HID), ("zT", BL), ("biases", 24),
]
PACK_OFF = {}
_o = 0
for _n, _w in _PACK_SPEC:
    PACK_OFF[_n] = (_o, _w)
    _o += _w
PACK_COLS = _o


def _build_graph(n_steps=NSTEPS, zero_bias=True, fp8_embed=True, hybrid_n=True):
    import concourse.bass as bass
    import concourse.bacc as bacc
    import concourse.mybir as mybir
    import concourse.tile as tile

    F32 = mybir.dt.float32
    BF16 = mybir.dt.bfloat16
    FP8 = mybir.dt.float8e4
    DR = mybir.MatmulPerfMode.DoubleRow
    AF = mybir.ActivationFunctionType
    OP = mybir.AluOpType

    nc = bacc.Bacc()

    if fp8_embed:
        oh_d = nc.declare_dram_parameter("oh", [n_steps, 74, 2 * BL], FP8,
                                         isOutput=False)
        # cols 0:2*G3 = fp8(emb2*64); cols 2*G3:4*G3 = fp8 of the residual
        # (emb2*64 - fp8(emb2*64)) - a second accumulating DoubleRow matmul
        # cancels ~94% of the fp8 quantisation error on the tanh path
        emb2dr_d = nc.declare_dram_parameter(
            "emb2dr", [74, (4 if hybrid_n else 2) * G3], FP8, isOutput=False)
    else:
        oh_d = nc.declare_dram_parameter("oh", [n_steps, VOCAB, BL], BF16,
                                         isOutput=False)
    wpack_d = nc.declare_dram_parameter("wpack", [128, PACK_COLS], BF16, isOutput=False)
    out_d = nc.declare_dram_parameter("out", [n_steps, VOCAB, BL], F32, isOutput=True)

    with tile.TileContext(nc) as tc:
        with (
            tc.tile_pool(name="const", bufs=1) as cpool,
            tc.tile_pool(name="io", bufs=6) as iopool,
            tc.tile_pool(name="work", bufs=2) as wpool,
            tc.tile_pool(name="psum", bufs=1, space="PSUM") as ppool,
        ):
            # ---- one DMA for every constant ----
            wpk = cpool.tile([128, PACK_COLS], BF16)
            nc.sync.dma_start(wpk[:], wpack_d[:, :])
            if fp8_embed:
                emb2dr = cpool.tile([74, (4 if hybrid_n else 2) * G3], FP8)
                nc.sync.dma_start(emb2dr[:], emb2dr_d[:, :])

            def P(name, rows=128):
                o, w = PACK_OFF[name]
                return wpk[0:rows, o:o + w]

            emb2a = P("emb2a")
            emb2b = P("emb2b", rows=VOCAB - 128)
            whh0 = [P("whh0k0"), P("whh0k1")]
            wih1 = [P("wih1k0"), P("wih1k1")]
            whh1 = [P("whh1k0"), P("whh1k1")]
            w1 = [P("w1k0"), P("w1k1")]
            w2 = [P("w2k0"), P("w2k1")]
            wlat = P("wlat", rows=LATENT)
            zT = P("zT", rows=LATENT)

            def bias_ap(col, rows=128):
                o, _ = PACK_OFF["biases"]
                return wpk[0:rows, o + col:o + col + 1]

            def mm(pt, lhsT, rhs, start, stop):
                nc.tensor.matmul(pt, lhsT, rhs, start=start, stop=stop)

            # ---- init hidden state: hT = W_lat @ zT + b_lat ----
            h0 = [None] * NCH   # wide [128, (k,512)] bf16 per chunk
            h1 = [None] * NCH
            for c in range(NCH):
                cs = slice(c * CH, (c + 1) * CH)
                h0[c] = wpool.tile([128, 2 * CH], BF16, tag="h0", bufs=6,
                                   name=f"h0i{c}")
                h1[c] = wpool.tile([128, 2 * CH], BF16, tag="h1", bufs=6,
                                   name=f"h1i{c}")
                for m in range(4):
                    ph = ppool.tile([128, CH], F32, tag="pn", bufs=4,
                                    name=f"pinit{c}_{m}")
                    mm(ph[:], wlat[:, m * 128:(m + 1) * 128], zT[:, cs],
                       True, True)
                    dst = (h0[c] if m < 2 else h1[c])
                    nc.scalar.activation(dst[:, (m % 2) * CH:(m % 2 + 1) * CH],
                                         ph[:], AF.Identity,
                                         bias=bias_ap(16 + m))

            def emit_head(t, h1s):
                for c in range(NCH):
                    cs = slice(c * CH, (c + 1) * CH)
                    hdd = wpool.tile([128, 2 * CH], BF16, tag="hdd", bufs=6,
                                     name=f"hdd{t}_{c}")
                    phds = []
                    for m in range(2):
                        ms = slice(m * 128, (m + 1) * 128)
                        phd = ppool.tile([128, CH], F32, tag="pr", bufs=2,
                                         name=f"phd{t}{c}{m}")
                        mm(phd[:], w1[0][:, ms], h1s[c][:, 0:CH], True, False)
                        mm(phd[:], w1[1][:, ms], h1s[c][:, CH:2 * CH], False, True)
                        phds.append(phd)
                    for m in range(2):
                        nc.scalar.activation(hdd[:, m * CH:(m + 1) * CH],
                                             phds[m][:], AF.Relu,
                                             bias=bias_ap(12 + m))
                    pl0 = ppool.tile([128, CH], F32, tag="pn", bufs=4,
                                     name=f"pl0{t}{c}")
                    mm(pl0[:], w2[0][:, 0:128], hdd[:, 0:CH], True, False)
                    mm(pl0[:], w2[1][:, 0:128], hdd[:, CH:2 * CH], False, True)
                    pl1 = ppool.tile([VOCAB - 128, CH], F32, tag="pn", bufs=4,
                                     name=f"pl1{t}{c}")
                    mm(pl1[:], w2[0][:, 128:VOCAB], hdd[:, 0:CH], True, False)
                    mm(pl1[:], w2[1][:, 128:VOCAB], hdd[:, CH:2 * CH], False, True)
                    lg0 = iopool.tile([128, CH], F32, tag="lg0",
                                      name=f"lg0{t}{c}")
                    lg1 = iopool.tile([VOCAB - 128, CH], F32, tag="lg1",
                                      name=f"lg1{t}{c}")
                    nc.scalar.activation(lg0[:], pl0[:], AF.Identity,
                                         bias=bias_ap(14))
                    nc.scalar.activation(lg1[:], pl1[:], AF.Identity,
                                         bias=bias_ap(15, rows=VOCAB - 128))
                    nc.sync.dma_start(out_d[t, 0:128, cs], lg0[:])
                    nc.sync.dma_start(out_d[t, 128:VOCAB, cs], lg1[:])

            pending_head = None

            # ---- time loop ----
            # Emission order = per-engine execution order. Emit chunk c's
            # matmuls, then its gate chain; chunk c+1's matmuls fill the PE
            # while chunk c's ACT/DVE chain runs. z-gate PSUM groups are
            # emitted last within a chunk (z is needed late) to cut peak
            # PSUM pressure.
            for t in range(n_steps):
                ohs = []
                for c in range(NCH):
                    cs = slice(c * CH, (c + 1) * CH)
                    if fp8_embed:
                        ohc = iopool.tile([74, 2 * CH], FP8, tag="oha",
                                          name=f"oh{t}_{c}")
                        nc.sync.dma_start(
                            ohc[:], oh_d[t].rearrange(
                                "k (j b) -> k j b", j=2)[:, :, cs])
                        ohs.append((ohc, None))
                    else:
                        oha = iopool.tile([128, CH], BF16, tag="oha",
                                          name=f"oha{t}_{c}")
                        nc.sync.dma_start(oha[:], oh_d[t, 0:128, cs])
                        ohb = iopool.tile([VOCAB - 128, CH], BF16, tag="ohb",
                                          name=f"ohb{t}_{c}")
                        nc.sync.dma_start(ohb[:], oh_d[t, 128:VOCAB, cs])
                        ohs.append((oha, ohb))

                h0new = [None] * NCH
                for layer in range(2):
                    if layer == 1 and pending_head is not None:
                        emit_head(*pending_head)
                        pending_head = None
                    if layer == 0:
                        wh = whh0
                        sigc, tanc, bhnc = 0, (4, 5), (20, 21)
                    else:
                        wh = whh1
                        sigc, tanc, bhnc = 6, (10, 11), (22, 23)

                    for c in range(NCH):
                        hprev = h0[c] if layer == 0 else h1[c]
                        use_dr = fp8_embed and layer == 0
                        if layer == 0:
                            if not fp8_embed:
                                ia, ib = emb2a, emb2b
                                ra, rb = ohs[c]
                            else:
                                oh_rhs = ohs[c][0].rearrange("k (j b) -> k j b", j=2)
                        else:
                            ia, ib = wih1[0], wih1[1]
                            ra = h0new[c][:, 0:CH]
                            rb = h0new[c][:, CH:2 * CH]

                        def mm_gi(pg, gs, start, stop, resid=False):
                            # gi contribution for gate rows gs
                            if use_dr:
                                main = emb2dr[:, 0:2 * G3].rearrange(
                                    "k (j m) -> k j m", j=2)[:, :, gs]
                                nc.tensor.matmul(pg, main, oh_rhs,
                                                 start=start,
                                                 stop=stop and not
                                                 (resid and hybrid_n),
                                                 perf_mode=DR)
                                if resid and hybrid_n:
                                    res = emb2dr[:, 2 * G3:4 * G3].rearrange(
                                        "k (j m) -> k j m", j=2)[:, :, gs]
                                    nc.tensor.matmul(pg, res, oh_rhs,
                                                     start=False, stop=stop,
                                                     perf_mode=DR)
                            else:
                                mm(pg, ia[:, gs], ra, start, False)
                                mm(pg, ib[:, gs], rb, False, stop)

                        def grp4(pg, gs):
                            mm(pg[:], wh[0][:, gs], hprev[:, 0:CH], True, False)
                            mm(pg[:], wh[1][:, gs], hprev[:, CH:2 * CH], False, False)
                            mm_gi(pg[:], gs, False, True)

                        # emission order: r (chain head), z, then the n-gate
                        # pairs - this ordering measured fastest end-to-end
                        pr, pin, phn, pz = [], [], [], []
                        for g in range(2):
                            pg = ppool.tile([128, CH], F32, tag="pr", bufs=2,
                                            name=f"pr{t}{c}{layer}{g}")
                            grp4(pg, slice(g * 128, (g + 1) * 128))
                            pr.append(pg)
                        for g in range(2):
                            pg = ppool.tile([128, CH], F32, tag="pz", bufs=2,
                                            name=f"pz{t}{c}{layer}{g}")
                            grp4(pg, slice((2 + g) * 128, (3 + g) * 128))
                            pz.append(pg)
                        for g in range(2):
                            gs = slice((4 + g) * 128, (5 + g) * 128)
                            pi = ppool.tile([128, CH], F32, tag="pn", bufs=4,
                                            name=f"pi{t}{c}{layer}{g}")
                            mm_gi(pi[:], gs, True, True, resid=True)
                            pin.append(pi)
                            pp = ppool.tile([128, CH], F32, tag="pn", bufs=4,
                                            name=f"pp{t}{c}{layer}{g}")
                            mm(pp[:], wh[0][:, gs], hprev[:, 0:CH], True, False)
                            mm(pp[:], wh[1][:, gs], hprev[:, CH:2 * CH], False, True)
                            phn.append(pp)
                        # ---- gate chain (ACT + DVE), in dependency order ----
                        rg, zg = [], []
                        for g in range(2):
                            r_ = wpool.tile([128, CH], BF16, tag="r", bufs=4,
                                            name=f"r{t}{c}{layer}{g}")
                            nc.scalar.activation(r_[:], pr[g][:], AF.Sigmoid,
                                                 bias=bias_ap(sigc + g))
                            rg.append(r_)
                        tmps, npres = [], []
                        for g in range(2):
                            tmp = wpool.tile([128, CH], BF16, tag="tmp", bufs=4,
                                             name=f"tm{t}{c}{layer}{g}")
                            if zero_bias:
                                nc.vector.tensor_mul(tmp[:], rg[g][:], phn[g][:])
                            else:
                                nc.vector.scalar_tensor_tensor(
                                    tmp[:], phn[g][:], bias_ap(bhnc[g]),
                                    rg[g][:], OP.add, OP.mult)
                            npre = wpool.tile([128, CH], BF16, tag="npre", bufs=4,
                                              name=f"np{t}{c}{layer}{g}")
                            nc.vector.tensor_add(npre[:], tmp[:], pin[g][:])
                            npres.append(npre)
                        for g in range(2):
                            z_ = wpool.tile([128, CH], BF16, tag="z", bufs=4,
                                            name=f"z{t}{c}{layer}{g}")
                            nc.scalar.activation(z_[:], pz[g][:], AF.Sigmoid,
                                                 bias=bias_ap(sigc + 2 + g))
                            zg.append(z_)
                        ns_ = []
                        for g in range(2):
                            n_ = wpool.tile([128, CH], BF16, tag="n", bufs=4,
                                            name=f"n{t}{c}{layer}{g}")
                            nc.scalar.activation(n_[:], npres[g][:], AF.Tanh,
                                                 bias=bias_ap(tanc[g]))
                            ns_.append(n_)
                        hn = wpool.tile([128, 2 * CH], BF16,
                                        tag=("h0" if layer == 0 else "h1"),
                                        bufs=6, name=f"h{layer}_{t}_{c}")
                        for g in range(2):
                            d_ = wpool.tile([128, CH], BF16, tag="d", bufs=4,
                                            name=f"d{t}{c}{layer}{g}")
                            nc.vector.tensor_sub(d_[:], hprev[:, g * CH:(g + 1) * CH],
                                                 ns_[g][:])
                            e_ = wpool.tile([128, CH], BF16, tag="e", bufs=4,
                                            name=f"e{t}{c}{layer}{g}")
                            nc.vector.tensor_mul(e_[:], zg[g][:], d_[:])
                            nc.vector.tensor_add(hn[:, g * CH:(g + 1) * CH],
                                                 ns_[g][:], e_[:])
                        if layer == 0:
                            h0new[c] = hn
                            h0[c] = hn
                        else:
                            h1[c] = hn

                pending_head = (t, [h1[0], h1[1]])
            if pending_head is not None:
                emit_head(*pending_head)
                pending_head = None

    nc.compile()
    return nc


def _host_prep(z, target_tokens, emb, W_lat, b_lat,
               W_ih0, W_hh0, b_ih0, b_hh0,
               W_ih1, W_hh1, b_ih1, b_hh1,
               W1, b1, W2, b2, n_steps=NSTEPS, fp8_embed=True, hybrid_n=True):
    """Build per-core input maps (all float32)."""
    f = np.float32
    z = np.asarray(z, f)
    tt = np.asarray(target_tokens)
    emb = np.asarray(emb, f)
    W_lat = np.asarray(W_lat, f)

    # teacher-forced input tokens: [START, tgt[:,1], ..., tgt[:,T-2]]
    tokens_in = np.concatenate(
        [np.full((B, 1), START_IDX, dtype=np.int64),
         np.asarray(tt[:, 1:T - 1], np.int64)], axis=1)  # [B, 49]
    tokens_in = tokens_in[:, :n_steps]

    emb2 = (emb @ np.asarray(W_ih0, f).T).astype(f)        # [VOCAB, 768]

    # bias packing: 24 columns
    bias = np.zeros((128, 24), f)
    b_ih0 = np.asarray(b_ih0, f); b_hh0 = np.asarray(b_hh0, f)
    b_ih1 = np.asarray(b_ih1, f); b_hh1 = np.asarray(b_hh1, f)
    sig0 = (b_ih0 + b_hh0)[:512].reshape(4, 128)
    sig1 = (b_ih1 + b_hh1)[:512].reshape(4, 128)
    for j in range(4):
        bias[:, j] = sig0[j]
        bias[:, 6 + j] = sig1[j]
    bias[:, 4] = b_ih0[512:640]; bias[:, 5] = b_ih0[640:768]
    bias[:, 10] = b_ih1[512:640]; bias[:, 11] = b_ih1[640:768]
    b1 = np.asarray(b1, f); b2 = np.asarray(b2, f)
    bias[:, 12] = b1[:128]; bias[:, 13] = b1[128:]
    bias[:, 14] = b2[:128]; bias[:VOCAB - 128, 15] = b2[128:]
    b_lat = np.asarray(b_lat, f)
    for j in range(4):
        bias[:, 16 + j] = b_lat[j * 128:(j + 1) * 128]
    bias[:, 20] = b_hh0[512:640]; bias[:, 21] = b_hh0[640:768]
    bias[:, 22] = b_hh1[512:640]; bias[:, 23] = b_hh1[640:768]

    import ml_dtypes
    bf16 = ml_dtypes.bfloat16
    wpack = np.zeros((128, PACK_COLS), bf16)

    def put(name, arr, rows=128):
        o, w = PACK_OFF[name]
        wpack[:rows, o:o + w] = arr.astype(bf16)

    whh0T = np.asarray(W_hh0, f).T
    wih1T = np.asarray(W_ih1, f).T
    whh1T = np.asarray(W_hh1, f).T
    w1T = np.asarray(W1, f).T
    w2T = np.asarray(W2, f).T
    put("emb2a", emb2[0:128])
    put("emb2b", emb2[128:VOCAB], rows=VOCAB - 128)
    put("whh0k0", whh0T[0:128]); put("whh0k1", whh0T[128:256])
    put("wih1k0", wih1T[0:128]); put("wih1k1", wih1T[128:256])
    put("whh1k0", whh1T[0:128]); put("whh1k1", whh1T[128:256])
    put("w1k0", w1T[0:128]); put("w1k1", w1T[128:256])
    put("w2k0", w2T[0:128]); put("w2k1", w2T[128:256])
    put("wlat", W_lat.T, rows=LATENT)
    put("biases", bias)

    if fp8_embed:
        import ml_dtypes as _md
        fp8 = _md.float8_e4m3
        # scale table up, one-hot down by an exact power of two: keeps the
        # product identical while lifting table entries out of fp8 subnormals
        e64 = np.zeros((74, 2, G3), np.float32)
        e64[:, 0, :] = emb2[0::2][:74]
        e64[:, 1, :] = emb2[1::2][:74]
        e64 = (e64 * 64.0).reshape(74, 2 * G3)
        main = e64.astype(fp8)
        if hybrid_n:
            res = (e64 - main.astype(np.float32)).astype(fp8)
            emb2dr = np.concatenate(
                [main.astype(np.float32), res.astype(np.float32)],
                axis=1).astype(fp8)
        else:
            emb2dr = main

    in_maps = []
    zo, zw = PACK_OFF["zT"]
    for core in range(NCORES):
        rows = slice(core * BL, (core + 1) * BL)
        tok = tokens_in[rows]                      # [BL, n_steps]
        tsteps = np.arange(n_steps)[None, :].repeat(BL, 0)   # [BL, n_steps]
        bidx = np.arange(BL)[:, None].repeat(n_steps, 1)
        wp = wpack.copy()
        wp[:LATENT, zo:zo + zw] = z[rows].T.astype(bf16)
        m = {"wpack": wp}
        if fp8_embed:
            # oh[t, ki, j*BL + b] = (tok[b,t] == 2*ki + j)
            oh = np.zeros((n_steps, 74, 2, BL), np.float32)
            oh[tsteps.ravel(), (tok // 2).ravel(), (tok % 2).ravel(),
               bidx.ravel()] = 1.0 / 64.0
            m["oh"] = oh.reshape(n_steps, 74, 2 * BL).astype(fp8)
            m["emb2dr"] = emb2dr
        else:
            oh = np.zeros((n_steps, VOCAB, BL), f)
            oh[tsteps.ravel(), tok.ravel(), bidx.ravel()] = 1.0
            m["oh"] = oh.astype(bf16)
        in_maps.append(m)
    return in_maps


class _Runner:
    """Compile once; run many times with device-resident inputs (no
    donation) so repeated calls time the NEFF execution itself."""

    def __init__(self, n_steps=NSTEPS, zero_bias=True, fp8_embed=True, hybrid_n=True):
        import jax
        import numpy as _np
        from jax.sharding import Mesh, PartitionSpec, NamedSharding
        from jax.experimental.shard_map import shard_map
        import concourse.bass2jax as b2j
        import concourse.mybir as mybir

        nc = _build_graph(n_steps, zero_bias=zero_bias, fp8_embed=fp8_embed,
                          hybrid_n=hybrid_n)
        self.fp8_embed = fp8_embed
        self.hybrid_n = hybrid_n
        b2j.install_neuronx_cc_hook()
        self.nc = nc
        self.n_steps = n_steps

        partition_name = (nc.partition_id_tensor.name
                          if nc.partition_id_tensor else None)
        in_names, out_names, out_avals, zero_outs = [], [], [], []
        for alloc in nc.m.functions[0].allocations:
            if not isinstance(alloc, mybir.MemoryLocationSet):
                continue
            name = alloc.memorylocations[0].name
            if alloc.kind == "ExternalInput":
                if name != partition_name:
                    in_names.append(name)
            elif alloc.kind == "ExternalOutput":
                shape = list(alloc.tensor_shape)
                out_avals.append(jax.core.ShapedArray(shape, _np.float32))
                out_names.append(name)
                zero_outs.append(_np.zeros(shape, _np.float32))
        self.in_names, self.out_names = list(in_names), out_names
        bind_names = list(in_names) + list(out_names)
        if partition_name is not None:
            bind_names.append(partition_name)

        def _body(*args):
            operands = list(args)
            if partition_name is not None:
                operands.append(b2j.partition_id_tensor())
            outs = b2j._bass_exec_p.bind(
                *operands,
                out_avals=tuple(out_avals),
                in_names=tuple(bind_names),
                out_names=tuple(out_names),
                lowering_input_output_aliases=(),
                sim_require_finite=True,
                sim_require_nnan=True,
                nc=nc,
            )
            return tuple(outs)

        devices = jax.devices()[:NCORES]
        mesh = Mesh(np.asarray(devices), ("core",))
        nin = len(in_names) + len(zero_outs)
        self._fn = jax.jit(shard_map(
            _body, mesh=mesh,
            in_specs=(PartitionSpec("core"),) * nin,
            out_specs=(PartitionSpec("core"),) * len(out_names),
            check_rep=False), keep_unused=True)
        self._sharding = NamedSharding(mesh, PartitionSpec("core"))
        self._jax = jax
        self._zero_outs = zero_outs
        self._placed = None

    def place(self, in_maps):
        """Transfer concatenated per-core inputs to the devices once."""
        jax = self._jax
        concat = []
        for name in self.in_names:
            arr = np.concatenate([m[name] for m in in_maps], axis=0)
            concat.append(jax.device_put(arr, self._sharding))
        for z in self._zero_outs:
            zz = np.zeros((NCORES * z.shape[0], *z.shape[1:]), z.dtype)
            concat.append(jax.device_put(zz, self._sharding))
        self._placed = concat

    def run(self):
        outs = self._fn(*self._placed)
        return outs

    def run_blocked(self):
        outs = self._fn(*self._placed)
        for o in outs:
            o.block_until_ready()
        return outs


def _assemble_logits(out_concat, n_steps):
    """out_concat: [NCORES*n_steps, VOCAB, BL] -> [B, n_steps, VOCAB]."""
    o = np.asarray(out_concat).reshape(NCORES, n_steps, VOCAB, BL)
    # [core, t, v, b] -> [core, b, t, v]
    return o.transpose(0, 3, 1, 2).reshape(B, n_steps, VOCAB)


def kernel(z, target_tokens, emb, W_lat, b_lat,
           W_ih0, W_hh0, b_ih0, b_hh0,
           W_ih1, W_hh1, b_ih1, b_hh1,
           W1, b1, W2, b2, _n_steps=NSTEPS, _runner=None):
    if _runner is None:
        zb = all(np.allclose(np.asarray(b), 0.0) for b in
                 (b_lat, b_ih0, b_hh0, b_ih1, b_hh1, b1, b2))
        _runner = _Runner(_n_steps, zero_bias=zb)
    r = _runner
    in_maps = _host_prep(z, target_tokens, emb, W_lat, b_lat,
                         W_ih0, W_hh0, b_ih0, b_hh0,
                         W_ih1, W_hh1, b_ih1, b_hh1,
                         W1, b1, W2, b2, n_steps=_n_steps,
                         fp8_embed=getattr(r, "fp8_embed", True),
                         hybrid_n=getattr(r, "hybrid_n", True))
    r.place(in_maps)
    outs = r.run_blocked()
    logits = _assemble_logits(outs[r.out_names.index("out")], _n_steps)
    generated = np.asarray(target_tokens)[:, 1:]
    return logits, generated


# revision 44
# speedup vs baseline: 105.5715x; 1.0053x over previous
"""Trainium2 Bass kernel for nn_AutoregressiveFormulaDecoder.

2-layer GRU decoder (HID=256) with teacher forcing + fused MLP head.
Pure data parallel over 8 NeuronCores: 1024 batch rows per core, no
collectives; host shards inputs and reassembles the output.

Device layout is "transposed" (features on SBUF partitions, batch on the
free dim) so weights are the PE-stationary operand and per-feature biases
are per-partition ACT biases.

Per step (49 steps), per 512-wide batch chunk:
  - the input-side projection gi0 = (emb @ W_ih0.T)[token] is computed as
    a one-hot matmul; for the r/z gates it runs in fp8e4m3 with
    perf_mode=DoubleRow (the 148-deep vocab contraction folds to one
    74-partition MM at 0.5 cyc/row; one-hot entries are exact in fp8 and
    the table/one-hot carry a 64 / 1-64 exact power-of-two rescale).
    The tanh-path i_n adds a SECOND accumulating DoubleRow matmul against
    an fp8 residual table fp8(emb2*64 - fp8(emb2*64)), cancelling ~94% of
    the fp8 quantisation error - accuracy matches the bf16 path.
  - r/z gates: gi and gh matmuls ACCUMULATE in one PSUM bank, then a
    single Sigmoid with fused per-partition bias reads PSUM directly.
  - n gate: i_n / h_n in separate PSUM banks; DVE r*h_n + i_n, Tanh.
  - h' = n + z*(h - n) as all-bf16 SBUF tensor ops (DVE 2x mode).
  - MLP head (relu(W1@h1), W2@...) is fused, software-pipelined one step
    behind the recurrence.

Scheduling notes (these drive the performance):
  - Tile engines execute their instruction streams IN EMISSION ORDER, so
    chunk c+1's matmuls are emitted between chunk c's matmuls and the
    ops that consume them - the PE stays busy while ACT/DVE run chunk
    c's gate chain.
  - PSUM tiles are tagged by drain class (pr=2, pn=4, pz=2 banks) so a
    new group's bank-reuse waits on an ACT/DVE op that actually fires
    early, not an arbitrary late one.
  - All constants ship in ONE packed bf16 DRAM tensor, split into a
    small "hot" DMA (wlat/z/biases -> init matmuls start ~7 us earlier)
    plus the bulk transfer: walrus allows at most 1 sync-wait per
    instruction, so fan-in from many DMA queues must be avoided.

Cost-model (CoreSim) predicted exec: ~1.026 ms for the full 49 steps
(PE-busy floor for this decomposition is ~1.006 ms -> 98% occupancy; the
ACT engine is at 0.965 ms, so this decomposition is near-saturated),
rel err ~6.3e-3 vs the float32 reference.
"""

import numpy as np

VOCAB = 148
START_IDX = 1
LATENT = 32
HID = 256
G3 = 3 * HID  # 768
B = 8192
T = 50
NSTEPS = T - 1  # 49
NCORES = 8
BL = B // NCORES  # 1024 batch rows per core
CH = 512          # batch chunk (one PSUM bank of f32)
NCH = BL // CH    # 2


# packed constant layout: name -> (col offset, col width); all float32 columns
_PACK_SPEC = [
    ("wlat", 2 * HID), ("zT", BL), ("biases", 24),   # "hot" init columns
    ("whh0k0", G3), ("whh0k1", G3),
    ("emb2a", G3), ("emb2b", G3),
    ("wih1k0", G3), ("wih1k1", G3),
    ("whh1k0", G3), ("whh1k1", G3),
    ("w1k0", HID), ("w1k1", HID),
    ("w2k0", VOCAB), ("w2k1", VOCAB),
]
HOT_COLS = 2 * HID + BL + 24
PACK_OFF = {}
_o = 0
for _n, _w in _PACK_SPEC:
    PACK_OFF[_n] = (_o, _w)
    _o += _w
PACK_COLS = _o


def _build_graph(n_steps=NSTEPS, zero_bias=True, fp8_embed=True, hybrid_n=True):
    import concourse.bass as bass
    import concourse.bacc as bacc
    import concourse.mybir as mybir
    import concourse.tile as tile

    F32 = mybir.dt.float32
    BF16 = mybir.dt.bfloat16
    FP8 = mybir.dt.float8e4
    DR = mybir.MatmulPerfMode.DoubleRow
    AF = mybir.ActivationFunctionType
    OP = mybir.AluOpType

    nc = bacc.Bacc()

    if fp8_embed:
        oh_d = nc.declare_dram_parameter("oh", [n_steps, 74, 2 * BL], FP8,
                                         isOutput=False)
        # cols 0:2*G3 = fp8(emb2*64); cols 2*G3:4*G3 = fp8 of the residual
        # (emb2*64 - fp8(emb2*64)) - a second accumulating DoubleRow matmul
        # cancels ~94% of the fp8 quantisation error on the tanh path
        emb2dr_d = nc.declare_dram_parameter(
            "emb2dr", [74, (4 if hybrid_n else 2) * G3], FP8, isOutput=False)
    else:
        oh_d = nc.declare_dram_parameter("oh", [n_steps, VOCAB, BL], BF16,
                                         isOutput=False)
    wpack_d = nc.declare_dram_parameter("wpack", [128, PACK_COLS], BF16, isOutput=False)
    out_d = nc.declare_dram_parameter("out", [n_steps, VOCAB, BL], F32, isOutput=True)

    with tile.TileContext(nc) as tc:
        with (
            tc.tile_pool(name="const", bufs=1) as cpool,
            tc.tile_pool(name="io", bufs=6) as iopool,
            tc.tile_pool(name="work", bufs=2) as wpool,
            tc.tile_pool(name="psum", bufs=1, space="PSUM") as ppool,
        ):
            # ---- one DMA for every constant ----
            # hot init columns first so the W_lat@z matmuls overlap the
            # bulk constant transfer
            wpk = cpool.tile([128, PACK_COLS], BF16)
            nc.sync.dma_start(wpk[:, 0:HOT_COLS], wpack_d[:, 0:HOT_COLS])
            nc.sync.dma_start(wpk[:, HOT_COLS:], wpack_d[:, HOT_COLS:])
            if fp8_embed:
                emb2dr = cpool.tile([74, (4 if hybrid_n else 2) * G3], FP8)
                nc.sync.dma_start(emb2dr[:], emb2dr_d[:, :])

            def P(name, rows=128):
                o, w = PACK_OFF[name]
                return wpk[0:rows, o:o + w]

            emb2a = P("emb2a")
            emb2b = P("emb2b", rows=VOCAB - 128)
            whh0 = [P("whh0k0"), P("whh0k1")]
            wih1 = [P("wih1k0"), P("wih1k1")]
            whh1 = [P("whh1k0"), P("whh1k1")]
            w1 = [P("w1k0"), P("w1k1")]
            w2 = [P("w2k0"), P("w2k1")]
            wlat = P("wlat", rows=LATENT)
            zT = P("zT", rows=LATENT)

            def bias_ap(col, rows=128):
                o, _ = PACK_OFF["biases"]
                return wpk[0:rows, o + col:o + col + 1]

            def mm(pt, lhsT, rhs, start, stop):
                nc.tensor.matmul(pt, lhsT, rhs, start=start, stop=stop)

            # ---- init hidden state: hT = W_lat @ zT + b_lat ----
            h0 = [None] * NCH   # wide [128, (k,512)] bf16 per chunk
            h1 = [None] * NCH
            for c in range(NCH):
                cs = slice(c * CH, (c + 1) * CH)
                h0[c] = wpool.tile([128, 2 * CH], BF16, tag="h0", bufs=6,
                                   name=f"h0i{c}")
                h1[c] = wpool.tile([128, 2 * CH], BF16, tag="h1", bufs=6,
                                   name=f"h1i{c}")
                for m in range(4):
                    ph = ppool.tile([128, CH], F32, tag="pn", bufs=4,
                                    name=f"pinit{c}_{m}")
                    mm(ph[:], wlat[:, m * 128:(m + 1) * 128], zT[:, cs],
                       True, True)
                    dst = (h0[c] if m < 2 else h1[c])
                    nc.scalar.activation(dst[:, (m % 2) * CH:(m % 2 + 1) * CH],
                                         ph[:], AF.Identity,
                                         bias=bias_ap(16 + m))

            def emit_head(t, h1s):
                for c in range(NCH):
                    cs = slice(c * CH, (c + 1) * CH)
                    hdd = wpool.tile([128, 2 * CH], BF16, tag="hdd", bufs=6,
                                     name=f"hdd{t}_{c}")
                    phds = []
                    for m in range(2):
                        ms = slice(m * 128, (m + 1) * 128)
                        phd = ppool.tile([128, CH], F32, tag="pr", bufs=2,
                                         name=f"phd{t}{c}{m}")
                        mm(phd[:], w1[0][:, ms], h1s[c][:, 0:CH], True, False)
                        mm(phd[:], w1[1][:, ms], h1s[c][:, CH:2 * CH], False, True)
                        phds.append(phd)
                    for m in range(2):
                        nc.scalar.activation(hdd[:, m * CH:(m + 1) * CH],
                                             phds[m][:], AF.Relu,
                                             bias=bias_ap(12 + m))
                    pl0 = ppool.tile([128, CH], F32, tag="pn", bufs=4,
                                     name=f"pl0{t}{c}")
                    mm(pl0[:], w2[0][:, 0:128], hdd[:, 0:CH], True, False)
                    mm(pl0[:], w2[1][:, 0:128], hdd[:, CH:2 * CH], False, True)
                    pl1 = ppool.tile([VOCAB - 128, CH], F32, tag="pn", bufs=4,
                                     name=f"pl1{t}{c}")
                    mm(pl1[:], w2[0][:, 128:VOCAB], hdd[:, 0:CH], True, False)
                    mm(pl1[:], w2[1][:, 128:VOCAB], hdd[:, CH:2 * CH], False, True)
                    lg0 = iopool.tile([128, CH], F32, tag="lg0",
                                      name=f"lg0{t}{c}")
                    lg1 = iopool.tile([VOCAB - 128, CH], F32, tag="lg1",
                                      name=f"lg1{t}{c}")
                    nc.scalar.activation(lg0[:], pl0[:], AF.Identity,
                                         bias=bias_ap(14))
                    nc.scalar.activation(lg1[:], pl1[:], AF.Identity,
                                         bias=bias_ap(15, rows=VOCAB - 128))
                    nc.sync.dma_start(out_d[t, 0:128, cs], lg0[:])
                    nc.sync.dma_start(out_d[t, 128:VOCAB, cs], lg1[:])

            pending_head = None

            # ---- time loop ----
            # Emission order = per-engine execution order. Emit chunk c's
            # matmuls, then its gate chain; chunk c+1's matmuls fill the PE
            # while chunk c's ACT/DVE chain runs. z-gate PSUM groups are
            # emitted last within a chunk (z is needed late) to cut peak
            # PSUM pressure.
            for t in range(n_steps):
                ohs = []
                for c in range(NCH):
                    cs = slice(c * CH, (c + 1) * CH)
                    if fp8_embed:
                        ohc = iopool.tile([74, 2 * CH], FP8, tag="oha",
                                          name=f"oh{t}_{c}")
                        nc.sync.dma_start(
                            ohc[:], oh_d[t].rearrange(
                                "k (j b) -> k j b", j=2)[:, :, cs])
                        ohs.append((ohc, None))
                    else:
                        oha = iopool.tile([128, CH], BF16, tag="oha",
                                          name=f"oha{t}_{c}")
                        nc.sync.dma_start(oha[:], oh_d[t, 0:128, cs])
                        ohb = iopool.tile([VOCAB - 128, CH], BF16, tag="ohb",
                                          name=f"ohb{t}_{c}")
                        nc.sync.dma_start(ohb[:], oh_d[t, 128:VOCAB, cs])
                        ohs.append((oha, ohb))

                h0new = [None] * NCH
                for layer in range(2):
                    if layer == 1 and pending_head is not None:
                        emit_head(*pending_head)
                        pending_head = None
                    if layer == 0:
                        wh = whh0
                        sigc, tanc, bhnc = 0, (4, 5), (20, 21)
                    else:
                        wh = whh1
                        sigc, tanc, bhnc = 6, (10, 11), (22, 23)

                    for c in range(NCH):
                        hprev = h0[c] if layer == 0 else h1[c]
                        use_dr = fp8_embed and layer == 0
                        if layer == 0:
                            if not fp8_embed:
                                ia, ib = emb2a, emb2b
                                ra, rb = ohs[c]
                            else:
                                oh_rhs = ohs[c][0].rearrange("k (j b) -> k j b", j=2)
                        else:
                            ia, ib = wih1[0], wih1[1]
                            ra = h0new[c][:, 0:CH]
                            rb = h0new[c][:, CH:2 * CH]

                        def mm_gi(pg, gs, start, stop, resid=False):
                            # gi contribution for gate rows gs
                            if use_dr:
                                main = emb2dr[:, 0:2 * G3].rearrange(
                                    "k (j m) -> k j m", j=2)[:, :, gs]
                                nc.tensor.matmul(pg, main, oh_rhs,
                                                 start=start,
                                                 stop=stop and not
                                                 (resid and hybrid_n),
                                                 perf_mode=DR)
                                if resid and hybrid_n:
                                    res = emb2dr[:, 2 * G3:4 * G3].rearrange(
                                        "k (j m) -> k j m", j=2)[:, :, gs]
                                    nc.tensor.matmul(pg, res, oh_rhs,
                                                     start=False, stop=stop,
                                                     perf_mode=DR)
                            else:
                                mm(pg, ia[:, gs], ra, start, False)
                                mm(pg, ib[:, gs], rb, False, stop)

                        def grp4(pg, gs):
                            mm(pg[:], wh[0][:, gs], hprev[:, 0:CH], True, False)
                            mm(pg[:], wh[1][:, gs], hprev[:, CH:2 * CH], False, False)
                            mm_gi(pg[:], gs, False, True)

                        # emission order: r (chain head), z, then the n-gate
                        # pairs - this ordering measured fastest end-to-end
                        pr, pin, phn, pz = [], [], [], []
                        for g in range(2):
                            pg = ppool.tile([128, CH], F32, tag="pr", bufs=2,
                                            name=f"pr{t}{c}{layer}{g}")
                            grp4(pg, slice(g * 128, (g + 1) * 128))
                            pr.append(pg)
                        for g in range(2):
                            pg = ppool.tile([128, CH], F32, tag="pz", bufs=2,
                                            name=f"pz{t}{c}{layer}{g}")
                            grp4(pg, slice((2 + g) * 128, (3 + g) * 128))
                            pz.append(pg)
                        for g in range(2):
                            gs = slice((4 + g) * 128, (5 + g) * 128)
                            pi = ppool.tile([128, CH], F32, tag="pn", bufs=4,
                                            name=f"pi{t}{c}{layer}{g}")
                            mm_gi(pi[:], gs, True, True, resid=True)
                            pin.append(pi)
                            pp = ppool.tile([128, CH], F32, tag="pn", bufs=4,
                                            name=f"pp{t}{c}{layer}{g}")
                            mm(pp[:], wh[0][:, gs], hprev[:, 0:CH], True, False)
                            mm(pp[:], wh[1][:, gs], hprev[:, CH:2 * CH], False, True)
                            phn.append(pp)
                        # ---- gate chain (ACT + DVE), in dependency order ----
                        rg, zg = [], []
                        for g in range(2):
                            r_ = wpool.tile([128, CH], BF16, tag="r", bufs=4,
                                            name=f"r{t}{c}{layer}{g}")
                            nc.scalar.activation(r_[:], pr[g][:], AF.Sigmoid,
                                                 bias=bias_ap(sigc + g))
                            rg.append(r_)
                        tmps, npres = [], []
                        for g in range(2):
                            tmp = wpool.tile([128, CH], BF16, tag="tmp", bufs=4,
                                             name=f"tm{t}{c}{layer}{g}")
                            if zero_bias:
                                nc.vector.tensor_mul(tmp[:], rg[g][:], phn[g][:])
                            else:
                                nc.vector.scalar_tensor_tensor(
                                    tmp[:], phn[g][:], bias_ap(bhnc[g]),
                                    rg[g][:], OP.add, OP.mult)
                            npre = wpool.tile([128, CH], BF16, tag="npre", bufs=4,
                                              name=f"np{t}{c}{layer}{g}")
                            nc.vector.tensor_add(npre[:], tmp[:], pin[g][:])
                            npres.append(npre)
                        for g in range(2):
                            z_ = wpool.tile([128, CH], BF16, tag="z", bufs=4,
                                            name=f"z{t}{c}{layer}{g}")
                            nc.scalar.activation(z_[:], pz[g][:], AF.Sigmoid,
                                                 bias=bias_ap(sigc + 2 + g))
                            zg.append(z_)
                        ns_ = []
                        for g in range(2):
                            n_ = wpool.tile([128, CH], BF16, tag="n", bufs=4,
                                            name=f"n{t}{c}{layer}{g}")
                            nc.scalar.activation(n_[:], npres[g][:], AF.Tanh,
                                                 bias=bias_ap(tanc[g]))
                            ns_.append(n_)
                        hn = wpool.tile([128, 2 * CH], BF16,
                                        tag=("h0" if layer == 0 else "h1"),
                                        bufs=6, name=f"h{layer}_{t}_{c}")
                        for g in range(2):
                            d_ = wpool.tile([128, CH], BF16, tag="d", bufs=4,
                                            name=f"d{t}{c}{layer}{g}")
                            nc.vector.tensor_sub(d_[:], hprev[:, g * CH:(g + 1) * CH],
                                                 ns_[g][:])
                            e_ = wpool.tile([128, CH], BF16, tag="e", bufs=4,
                                            name=f"e{t}{c}{layer}{g}")
                            nc.vector.tensor_mul(e_[:], zg[g][:], d_[:])
                            nc.vector.tensor_add(hn[:, g * CH:(g + 1) * CH],
                                                 ns_[g][:], e_[:])
                        if layer == 0:
                            h0new[c] = hn
                            h0[c] = hn
                        else:
                            h1[c] = hn

                pending_head = (t, [h1[0], h1[1]])
            if pending_head is not None:
                emit_head(*pending_head)
                pending_head = None

    nc.compile()
    return nc


def _host_prep(z, target_tokens, emb, W_lat, b_lat,
               W_ih0, W_hh0, b_ih0, b_hh0,
               W_ih1, W_hh1, b_ih1, b_hh1,
               W1, b1, W2, b2, n_steps=NSTEPS, fp8_embed=True, hybrid_n=True):
    """Build per-core input maps (all float32)."""
    f = np.float32
    z = np.asarray(z, f)
    tt = np.asarray(target_tokens)
    emb = np.asarray(emb, f)
    W_lat = np.asarray(W_lat, f)

    # teacher-forced input tokens: [START, tgt[:,1], ..., tgt[:,T-2]]
    tokens_in = np.concatenate(
        [np.full((B, 1), START_IDX, dtype=np.int64),
         np.asarray(tt[:, 1:T - 1], np.int64)], axis=1)  # [B, 49]
    tokens_in = tokens_in[:, :n_steps]

    emb2 = (emb @ np.asarray(W_ih0, f).T).astype(f)        # [VOCAB, 768]

    # bias packing: 24 columns
    bias = np.zeros((128, 24), f)
    b_ih0 = np.asarray(b_ih0, f); b_hh0 = np.asarray(b_hh0, f)
    b_ih1 = np.asarray(b_ih1, f); b_hh1 = np.asarray(b_hh1, f)
    sig0 = (b_ih0 + b_hh0)[:512].reshape(4, 128)
    sig1 = (b_ih1 + b_hh1)[:512].reshape(4, 128)
    for j in range(4):
        bias[:, j] = sig0[j]
        bias[:, 6 + j] = sig1[j]
    bias[:, 4] = b_ih0[512:640]; bias[:, 5] = b_ih0[640:768]
    bias[:, 10] = b_ih1[512:640]; bias[:, 11] = b_ih1[640:768]
    b1 = np.asarray(b1, f); b2 = np.asarray(b2, f)
    bias[:, 12] = b1[:128]; bias[:, 13] = b1[128:]
    bias[:, 14] = b2[:128]; bias[:VOCAB - 128, 15] = b2[128:]
    b_lat = np.asarray(b_lat, f)
    for j in range(4):
        bias[:, 16 + j] = b_lat[j * 128:(j + 1) * 128]
    bias[:, 20] = b_hh0[512:640]; bias[:, 21] = b_hh0[640:768]
    bias[:, 22] = b_hh1[512:640]; bias[:, 23] = b_hh1[640:768]

    import ml_dtypes
    bf16 = ml_dtypes.bfloat16
    wpack = np.zeros((128, PACK_COLS), bf16)

    def put(name, arr, rows=128):
        o, w = PACK_OFF[name]
        wpack[:rows, o:o + w] = arr.astype(bf16)

    whh0T = np.asarray(W_hh0, f).T
    wih1T = np.asarray(W_ih1, f).T
    whh1T = np.asarray(W_hh1, f).T
    w1T = np.asarray(W1, f).T
    w2T = np.asarray(W2, f).T
    put("emb2a", emb2[0:128])
    put("emb2b", emb2[128:VOCAB], rows=VOCAB - 128)
    put("whh0k0", whh0T[0:128]); put("whh0k1", whh0T[128:256])
    put("wih1k0", wih1T[0:128]); put("wih1k1", wih1T[128:256])
    put("whh1k0", whh1T[0:128]); put("whh1k1", whh1T[128:256])
    put("w1k0", w1T[0:128]); put("w1k1", w1T[128:256])
    put("w2k0", w2T[0:128]); put("w2k1", w2T[128:256])
    put("wlat", W_lat.T, rows=LATENT)
    put("biases", bias)

    if fp8_embed:
        import ml_dtypes as _md
        fp8 = _md.float8_e4m3
        # scale table up, one-hot down by an exact power of two: keeps the
        # product identical while lifting table entries out of fp8 subnormals
        e64 = np.zeros((74, 2, G3), np.float32)
        e64[:, 0, :] = emb2[0::2][:74]
        e64[:, 1, :] = emb2[1::2][:74]
        e64 = (e64 * 64.0).reshape(74, 2 * G3)
        main = e64.astype(fp8)
        if hybrid_n:
            res = (e64 - main.astype(np.float32)).astype(fp8)
            emb2dr = np.concatenate(
                [main.astype(np.float32), res.astype(np.float32)],
                axis=1).astype(fp8)
        else:
            emb2dr = main

    in_maps = []
    zo, zw = PACK_OFF["zT"]
    for core in range(NCORES):
        rows = slice(core * BL, (core + 1) * BL)
        tok = tokens_in[rows]                      # [BL, n_steps]
        tsteps = np.arange(n_steps)[None, :].repeat(BL, 0)   # [BL, n_steps]
        bidx = np.arange(BL)[:, None].repeat(n_steps, 1)
        wp = wpack.copy()
        wp[:LATENT, zo:zo + zw] = z[rows].T.astype(bf16)
        m = {"wpack": wp}
        if fp8_embed:
            # oh[t, ki, j*BL + b] = (tok[b,t] == 2*ki + j)
            oh = np.zeros((n_steps, 74, 2, BL), np.float32)
            oh[tsteps.ravel(), (tok // 2).ravel(), (tok % 2).ravel(),
               bidx.ravel()] = 1.0 / 64.0
            m["oh"] = oh.reshape(n_steps, 74, 2 * BL).astype(fp8)
            m["emb2dr"] = emb2dr
        else:
            oh = np.zeros((n_steps, VOCAB, BL), f)
            oh[tsteps.ravel(), tok.ravel(), bidx.ravel()] = 1.0
            m["oh"] = oh.astype(bf16)
        in_maps.append(m)
    return in_maps


class _Runner:
    """Compile once; run many times with device-resident inputs (no
    donation) so repeated calls time the NEFF execution itself."""

    def __init__(self, n_steps=NSTEPS, zero_bias=True, fp8_embed=True, hybrid_n=True):
        import jax
        import numpy as _np
        from jax.sharding import Mesh, PartitionSpec, NamedSharding
        from jax.experimental.shard_map import shard_map
        import concourse.bass2jax as b2j
        import concourse.mybir as mybir

        nc = _build_graph(n_steps, zero_bias=zero_bias, fp8_embed=fp8_embed,
                          hybrid_n=hybrid_n)
        self.fp8_embed = fp8_embed
        self.hybrid_n = hybrid_n
        b2j.install_neuronx_cc_hook()
        self.nc = nc
        self.n_steps = n_steps

        partition_name = (nc.partition_id_tensor.name
                          if nc.partition_id_tensor else None)
        in_names, out_names, out_avals, zero_outs = [], [], [], []
        for alloc in nc.m.functions[0].allocations:
            if not isinstance(alloc, mybir.MemoryLocationSet):
                continue
            name = alloc.memorylocations[0].name
            if alloc.kind == "ExternalInput":
                if name != partition_name:
                    in_names.append(name)
            elif alloc.kind == "ExternalOutput":
                shape = list(alloc.tensor_shape)
                out_avals.append(jax.core.ShapedArray(shape, _np.float32))
                out_names.append(name)
                zero_outs.append(_np.zeros(shape, _np.float32))
        self.in_names, self.out_names = list(in_names), out_names
        bind_names = list(in_names) + list(out_names)
        if partition_name is not None:
            bind_names.append(partition_name)

        def _body(*args):
            operands = list(args)
            if partition_name is not None:
                operands.append(b2j.partition_id_tensor())
            outs = b2j._bass_exec_p.bind(
                *operands,
                out_avals=tuple(out_avals),
                in_names=tuple(bind_names),
                out_names=tuple(out_names),
                lowering_input_output_aliases=(),
                sim_require_finite=True,
                sim_require_nnan=True,
                nc=nc,
            )
            return tuple(outs)

        devices = jax.devices()[:NCORES]
        mesh = Mesh(np.asarray(devices), ("core",))
        nin = len(in_names) + len(zero_outs)
        self._fn = jax.jit(shard_map(
            _body, mesh=mesh,
            in_specs=(PartitionSpec("core"),) * nin,
            out_specs=(PartitionSpec("core"),) * len(out_names),
            check_rep=False), keep_unused=True)
        self._sharding = NamedSharding(mesh, PartitionSpec("core"))
        self._jax = jax
        self._zero_outs = zero_outs
        self._placed = None

    def place(self, in_maps):
        """Transfer concatenated per-core inputs to the devices once."""
        jax = self._jax
        concat = []
        for name in self.in_names:
            arr = np.concatenate([m[name] for m in in_maps], axis=0)
            concat.append(jax.device_put(arr, self._sharding))
        for z in self._zero_outs:
            zz = np.zeros((NCORES * z.shape[0], *z.shape[1:]), z.dtype)
            concat.append(jax.device_put(zz, self._sharding))
        self._placed = concat

    def run(self):
        outs = self._fn(*self._placed)
        return outs

    def run_blocked(self):
        outs = self._fn(*self._placed)
        for o in outs:
            o.block_until_ready()
        return outs


def _assemble_logits(out_concat, n_steps):
    """out_concat: [NCORES*n_steps, VOCAB, BL] -> [B, n_steps, VOCAB]."""
    o = np.asarray(out_concat).reshape(NCORES, n_steps, VOCAB, BL)
    # [core, t, v, b] -> [core, b, t, v]
    return o.transpose(0, 3, 1, 2).reshape(B, n_steps, VOCAB)


def kernel(z, target_tokens, emb, W_lat, b_lat,
           W_ih0, W_hh0, b_ih0, b_hh0,
           W_ih1, W_hh1, b_ih1, b_hh1,
           W1, b1, W2, b2, _n_steps=NSTEPS, _runner=None):
    if _runner is None:
        zb = all(np.allclose(np.asarray(b), 0.0) for b in
                 (b_lat, b_ih0, b_hh0, b_ih1, b_hh1, b1, b2))
        _runner = _Runner(_n_steps, zero_bias=zb)
    r = _runner
    in_maps = _host_prep(z, target_tokens, emb, W_lat, b_lat,
                         W_ih0, W_hh0, b_ih0, b_hh0,
                         W_ih1, W_hh1, b_ih1, b_hh1,
                         W1, b1, W2, b2, n_steps=_n_steps,
                         fp8_embed=getattr(r, "fp8_embed", True),
                         hybrid_n=getattr(r, "hybrid_n", True))
    r.place(in_maps)
    outs = r.run_blocked()
    logits = _assemble_logits(outs[r.out_names.index("out")], _n_steps)
    generated = np.asarray(target_tokens)[:, 1:]
    return logits, generated
